# revision 1
# baseline (speedup 1.0000x reference)
"""AxialAttention (vertical, cls token, full cls attention) Trainium2 kernel.

Strategy: data-parallel over batch (32 batches -> 8 cores x 4 batches).
Per core everything is done in one fused Bass/Tile program:
  qkv projection -> per-row axial attention (+ full cls attention) -> out proj.

Host-side layout (per batch element):
  tokens are regrouped into 7 "slots" of 113 columns: [cls, 4 rows x 28 tok].
  Everything on-chip is feature-major (C on partitions): x_T (768, 791*4).
  Row attention for slot g, head h:
    scores[112 q, 113 k] = (qT slice).T @ (kT slice) + mask-matmul (row-match
    term: +30 same-row/cls, 0 otherwise), exp(bias=-30) with accum -> Z,
    U = exp * (1/Z) (per-partition broadcast), PE-transpose -> U_T,
    Y = (v_slot).T @ U_T accumulated per chunk of 128 output features.
  cls attention: per-head logits computed transposed ([keys, head] per slot),
    exp, dup-cls rows zeroed, Z via ones-matmul, 1/Z broadcast via K=1 matmul,
    v.T @ u accumulated over slots.
  proj: out_T = W_proj.T-contract with Y_T (+ bias via ACT Identity bias).
"""

import os

os.environ.setdefault("JAX_PLATFORMS", "axon")

import sys

if "/opt/trn_rl_repo" not in sys.path:
    sys.path.insert(0, "/opt/trn_rl_repo")

import numpy as np
import ml_dtypes

import concourse.bass as bass
import concourse.bacc as bacc
import concourse.mybir as mybir
import concourse.tile as tile
from concourse.bass_utils import run_bass_kernel_spmd
from concourse.masks import make_identity

P = 128
C = 768
CH = C // P            # 6 feature chunks
NH = 12
HD = 64
HH = 28                # image H = W
ROWS = 28              # attention rows per image
RG = 4                 # rows per slot
G = 7                  # slots per batch
W = RG * HH            # 112 queries per slot
SC = W + 1             # 113 keys per slot (cls + tokens)
S = G * SC             # 791 columns per batch
NB = 4                 # batches per core
TT = NB * S            # 3164 columns per core
NCORES = 8
B_TOTAL = 32
N_TOK = 1 + ROWS * HH  # 785
MPEN = 30.0            # mask penalty

F32 = mybir.dt.float32
BF16 = mybir.dt.bfloat16
BFNP = ml_dtypes.bfloat16


def _perm_valid():
    """original-token index for each of the S slot-layout columns + validity."""
    perm = np.zeros(S, np.int64)
    valid = np.ones(S, np.bool_)
    for g in range(G):
        perm[g * SC] = 0
        if g > 0:
            valid[g * SC] = False
        for j in range(W):
            r = RG * g + j // HH   # row index (original column w)
            i = j % HH             # position in row (original row h)
            perm[g * SC + 1 + j] = 1 + i * HH + r
    return perm, valid


def _consts():
    rt = np.sqrt(MPEN)
    qext = np.zeros((RG, W), np.float32)
    for j in range(W):
        qext[j // HH, j] = rt
    kext = np.zeros((RG, SC), np.float32)
    kext[:, 0] = rt
    for j in range(W):
        kext[j // HH, 1 + j] = rt
    return qext.astype(BFNP), kext.astype(BFNP)


def build_bass():
    nc = bacc.Bacc(None, target_bir_lowering=False, debug=True)

    x_t = nc.declare_dram_parameter("x_t", [C, TT], BF16, isOutput=False)
    w_qkv = nc.declare_dram_parameter("w_qkv", [C, 3 * C], BF16, isOutput=False)
    w_proj = nc.declare_dram_parameter("w_proj", [C, C], BF16, isOutput=False)
    b_pc = nc.declare_dram_parameter("b_pc", [P, CH], F32, isOutput=False)
    qext = nc.declare_dram_parameter("qext", [RG, W], BF16, isOutput=False)
    kext = nc.declare_dram_parameter("kext", [RG, SC], BF16, isOutput=False)
    out_t = nc.declare_dram_parameter("out_t", [C, TT], F32, isOutput=True)

    NSPLIT = [(0, 512), (512, S - 512)]          # moving-dim tiling of S
    VSPLIT = [(0, 512), (512, C - 512)]          # moving-dim tiling of C (v cols)

    with tile.TileContext(nc) as tc:
        with (
            tc.tile_pool(name="const", bufs=1) as cpool,
            tc.tile_pool(name="xb", bufs=2) as xpool,
            tc.tile_pool(name="qk", bufs=2) as qkpool,
            tc.tile_pool(name="vt", bufs=2) as vpool,
            tc.tile_pool(name="yt", bufs=2) as ypool,
            tc.tile_pool(name="ob", bufs=2) as opool,
            tc.tile_pool(name="att", bufs=3) as apool,
            tc.tile_pool(name="attz", bufs=2) as zpool,
            tc.tile_pool(name="ps_mm", bufs=2, space="PSUM") as ps_mm,
            tc.tile_pool(name="ps_sc", bufs=2, space="PSUM") as ps_sc,
            tc.tile_pool(name="ps_tp", bufs=2, space="PSUM") as ps_tp,
            tc.tile_pool(name="ps_y", bufs=2, space="PSUM") as ps_y,
        ):
            # ---- constants ----
            wq_sb = cpool.tile([P, CH, 3 * C], BF16)
            nc.sync.dma_start(wq_sb[:], w_qkv.rearrange("(c p) o -> p c o", p=P))
            wp_sb = cpool.tile([P, CH, C], BF16)
            nc.sync.dma_start(wp_sb[:], w_proj.rearrange("(c p) o -> p c o", p=P))
            b_sb = cpool.tile([P, CH], F32)
            nc.sync.dma_start(b_sb[:], b_pc[:])
            qe_sb = cpool.tile([RG, W], BF16)
            nc.sync.dma_start(qe_sb[:], qext[:])
            ke_sb = cpool.tile([RG, SC], BF16)
            nc.sync.dma_start(ke_sb[:], kext[:])
            ident = cpool.tile([P, P], BF16)
            make_identity(nc, ident[:])
            onesp = cpool.tile([P, 1], BF16)
            nc.vector.memset(onesp[:], 1.0)
            onesrow = cpool.tile([1, P], F32)
            nc.vector.memset(onesrow[:], 1.0)
            negm = cpool.tile([P, 1], F32)
            nc.vector.memset(negm[:], -MPEN)

            for b in range(NB):
                cb = b * S
                # ---- load x_T for this batch ----
                xb = xpool.tile([P, CH, S], BF16, tag="xb")
                nc.sync.dma_start(
                    xb[:], x_t[:, cb : cb + S].rearrange("(c p) s -> p c s", p=P)
                )

                qT = qkpool.tile([P, CH, S], BF16, tag="qT")
                kT = qkpool.tile([P, CH, S], BF16, tag="kT")
                vtok = vpool.tile([P, G, C], BF16, tag="vtok")

                # ---- qkv projections ----
                # q and k: feature-major output [feat chunk, token]
                for o in range(2 * CH):
                    dst = qT if o < CH else kT
                    oc = o % CH
                    for (n0, nsz) in NSPLIT:
                        ps = ps_mm.tile([P, 512], F32, tag="mm")
                        for c in range(CH):
                            nc.tensor.matmul(
                                ps[:, :nsz],
                                lhsT=wq_sb[:, c, o * P : (o + 1) * P],
                                rhs=xb[:, c, n0 : n0 + nsz],
                                start=(c == 0),
                                stop=(c == CH - 1),
                            )
                        nc.scalar.copy(dst[:, oc, n0 : n0 + nsz], ps[:, :nsz])
                # v: token-major per slot [113 tokens, C]
                for g in range(G):
                    for (v0, vsz) in VSPLIT:
                        ps = ps_mm.tile([P, 512], F32, tag="mm")
                        for c in range(CH):
                            nc.tensor.matmul(
                                ps[:SC, :vsz],
                                lhsT=xb[:, c, g * SC : (g + 1) * SC],
                                rhs=wq_sb[:, c, 2 * C + v0 : 2 * C + v0 + vsz],
                                start=(c == 0),
                                stop=(c == CH - 1),
                            )
                        nc.scalar.copy(
                            vtok[:SC, g, v0 : v0 + vsz], ps[:SC, :vsz]
                        )

                yT = ypool.tile([P, CH, S], BF16, tag="yT")
                nc.vector.memset(yT[:], 0.0)

                # ---- row attention ----
                for g in range(G):
                    k0 = g * SC
                    zt = zpool.tile([W, NH], F32, tag="zt")
                    rzt = zpool.tile([W, NH], F32, tag="rzt")
                    us = []
                    for h in range(NH):
                        c, hf = h // 2, (h % 2) * HD
                        sp = ps_sc.tile([W, SC], F32, tag="sc")
                        nc.tensor.matmul(
                            sp[:],
                            lhsT=qT[hf : hf + HD, c, k0 + 1 : k0 + SC],
                            rhs=kT[hf : hf + HD, c, k0 : k0 + SC],
                            start=True,
                            stop=False,
                        )
                        nc.tensor.matmul(
                            sp[:],
                            lhsT=qe_sb[:],
                            rhs=ke_sb[:],
                            start=False,
                            stop=True,
                        )
                        u = apool.tile([W, SC], BF16, tag="u", bufs=14)
                        nc.scalar.activation(
                            u[:],
                            sp[:],
                            mybir.ActivationFunctionType.Exp,
                            bias=negm[:W, :],
                            accum_out=zt[:, h : h + 1],
                        )
                        # per-head reciprocal + normalize: subtile deps let each
                        # head's chain complete without waiting for all 12 exps
                        nc.vector.reciprocal(rzt[:, h : h + 1], zt[:, h : h + 1])
                        nc.vector.tensor_tensor(
                            u[:],
                            u[:],
                            rzt[:, h : h + 1].to_broadcast([W, SC]),
                            mybir.AluOpType.mult,
                        )
                        us.append(u)
                    uts = []
                    for h in range(NH):
                        tp = ps_tp.tile([SC, W], BF16, tag="tp")
                        nc.tensor.transpose(tp[:], us[h][:], ident[:W, :W])
                        ut = apool.tile([SC, W], BF16, tag="ut", bufs=14)
                        nc.vector.tensor_copy(ut[:], tp[:])
                        uts.append(ut)
                    for c in range(CH):
                        yp = ps_y.tile([P, W], F32, tag="y")
                        for hf in range(2):
                            h = 2 * c + hf
                            nc.tensor.matmul(
                                yp[hf * HD : (hf + 1) * HD, :],
                                lhsT=vtok[:SC, g, h * HD : (h + 1) * HD],
                                rhs=uts[h][:],
                                start=True,
                                stop=True,
                            )
                        nc.vector.tensor_copy(yT[:, c, k0 + 1 : k0 + SC], yp[:])

                # ---- cls attention (logits computed transposed: [keys, head]) ----
                ucts = []
                for g in range(G):
                    cp = ps_tp.tile([SC, NH], F32, tag="tp")
                    for h in range(NH):
                        c, hf = h // 2, (h % 2) * HD
                        nc.tensor.matmul(
                            cp[:, h : h + 1],
                            lhsT=kT[hf : hf + HD, c, g * SC : (g + 1) * SC],
                            rhs=qT[hf : hf + HD, c, 0:1],
                            start=True,
                            stop=True,
                        )
                    uct = apool.tile([SC, NH], BF16, tag="uct", bufs=9)
                    nc.scalar.activation(
                        uct[:], cp[:], mybir.ActivationFunctionType.Exp
                    )
                    if g > 0:
                        nc.vector.memset(uct[0:1, :], 0.0)
                    ucts.append(uct)
                # Z over all keys via ones-vector matmuls, accumulated per slot
                zp = ps_tp.tile([1, NH], F32, tag="tp")
                for g in range(G):
                    nc.tensor.matmul(
                        zp[:],
                        lhsT=onesp[:SC, :],
                        rhs=ucts[g][:],
                        start=(g == 0),
                        stop=(g == G - 1),
                    )
                zcs = zpool.tile([1, NH], F32, tag="zcs")
                nc.vector.tensor_copy(zcs[:], zp[:])
                rzc = zpool.tile([1, NH], F32, tag="rzc")
                nc.vector.reciprocal(rzc[:], zcs[:])
                # broadcast 1/Z to all partitions via a K=1 matmul with ones
                rzb = ps_tp.tile([P, NH], F32, tag="tp")
                nc.tensor.matmul(
                    rzb[:], lhsT=onesrow[:], rhs=rzc[:], start=True, stop=True
                )
                for g in range(G):
                    nc.vector.tensor_tensor(
                        ucts[g][:], ucts[g][:], rzb[:SC, :], mybir.AluOpType.mult
                    )
                for c in range(CH):
                    yp = ps_y.tile([P, W], F32, tag="y")
                    for hf in range(2):
                        h = 2 * c + hf
                        for g in range(G):
                            nc.tensor.matmul(
                                yp[hf * HD : (hf + 1) * HD, 0:1],
                                lhsT=vtok[:SC, g, h * HD : (h + 1) * HD],
                                rhs=ucts[g][:, h : h + 1],
                                start=(g == 0),
                                stop=(g == G - 1),
                            )
                    nc.vector.tensor_copy(yT[:, c, 0:1], yp[:, 0:1])

                # ---- output projection ----
                ob = opool.tile([P, CH, S], F32, tag="ob")
                for o in range(CH):
                    for (n0, nsz) in NSPLIT:
                        ps = ps_mm.tile([P, 512], F32, tag="mm")
                        for c in range(CH):
                            nc.tensor.matmul(
                                ps[:, :nsz],
                                lhsT=wp_sb[:, c, o * P : (o + 1) * P],
                                rhs=yT[:, c, n0 : n0 + nsz],
                                start=(c == 0),
                                stop=(c == CH - 1),
                            )
                        nc.scalar.activation(
                            ob[:, o, n0 : n0 + nsz],
                            ps[:, :nsz],
                            mybir.ActivationFunctionType.Identity,
                            bias=b_sb[:, o : o + 1],
                        )
                nc.sync.dma_start(
                    out_t[:, cb : cb + S].rearrange("(c p) s -> p c s", p=P), ob[:]
                )

    nc.compile()
    return nc


_NC_CACHE = None
_LAST_IN_MAPS = None


def kernel(x, w_qkv, w_proj, b_proj):
    global _NC_CACHE, _LAST_IN_MAPS
    x = np.asarray(x)
    w_qkv = np.asarray(w_qkv)
    w_proj = np.asarray(w_proj)
    b_proj = np.asarray(b_proj)

    perm, valid = _perm_valid()
    qext, kext = _consts()

    wq = np.array(w_qkv, np.float32, copy=True)
    wq[:, :C] *= 1.0 / np.sqrt(HD)
    wq = wq.astype(BFNP)
    wp = w_proj.astype(BFNP)
    b_pc = np.ascontiguousarray(b_proj.astype(np.float32).reshape(CH, P).T)

    in_maps = []
    for core in range(NCORES):
        xs = x[core * NB : (core + 1) * NB]          # (NB, 785, C)
        xp = xs[:, perm, :]                          # (NB, S, C)
        x_T = np.ascontiguousarray(
            xp.transpose(2, 0, 1).reshape(C, TT)
        ).astype(BFNP)
        in_maps.append(
            {
                "x_t": x_T,
                "w_qkv": wq,
                "w_proj": wp,
                "b_pc": b_pc,
                "qext": qext,
                "kext": kext,
            }
        )

    if _NC_CACHE is None:
        _NC_CACHE = build_bass()
    nc = _NC_CACHE

    _LAST_IN_MAPS = in_maps

    res = run_bass_kernel_spmd(nc, in_maps, core_ids=list(range(NCORES)))

    out = np.zeros((B_TOTAL, N_TOK, C), np.float32)
    vperm = perm[valid]
    for core in range(NCORES):
        o_t = res.results[core]["out_t"]             # (C, TT) f32
        op = o_t.reshape(C, NB, S).transpose(1, 2, 0)  # (NB, S, C)
        out[core * NB : (core + 1) * NB][:, vperm, :] = op[:, valid, :]
    return out


if __name__ == "__main__":
    rng = np.random.default_rng(0)
    x = rng.standard_normal((B_TOTAL, N_TOK, C)).astype(np.float32)
    w_qkv = (rng.standard_normal((C, 3 * C)) * 0.02).astype(np.float32)
    w_proj = (rng.standard_normal((C, C)) * 0.02).astype(np.float32)
    b_proj = np.zeros((C,), np.float32)
    y = kernel(x=x, w_qkv=w_qkv, w_proj=w_proj, b_proj=b_proj)
    print(y.shape, y.dtype)



# revision 23
# speedup vs baseline: 1.1176x; 1.1176x over previous
"""AxialAttention (vertical, cls token, full cls attention) Trainium2 kernel.

Strategy: data-parallel over batch (32 batches -> 8 cores x 4 batches).
Per core everything is done in one fused Bass/Tile program:
  qkv projection -> per-row axial attention (+ full cls attention) -> out proj.

Host-side layout (per batch element):
  tokens are regrouped into 7 "slots" of 113 columns: [cls, 4 rows x 28 tok].
  Everything on-chip is feature-major (C on partitions): x_T (768, 791*4).

Row attention computes scores TRANSPOSED (keys on partitions) so no PE
transpose of the attention weights is needed:
  scoresT[k, q] = k_slot.T @ q_slot per head, 4 heads per PSUM tile with one
  bank per head (HW requires one matmul group per 2KB PSUM bank), one exp per
  4-head tile straight to SBUF, one multiplicative-mask multiply (exp(s+m-30)
  == exp(s)*M with M=1 valid / e^-30 invalid), per-head Z via tiny ones-matmuls
  into a shared [112, 84] PSUM tile, reciprocal, and a per-(slot, chunk)
  PE-broadcast of 1/Z (eye2-matmul) applied at the yT writeback
  (tensor_tensor multiply instead of a copy).
cls attention result is broadcast into all 7 dup-cls columns of yT so no
yT memset is needed.

Emission is software-pipelined per batch so each engine gets long runs of
independent work: qk chunk-pairs interleaved with score tiles, the previous
batch's projection as dense cover, then a v-projection stretch carrying the
t=2 scores, the 1/Z chains and the AV+writeback with 1-2 slot lag.
"""

import os

os.environ.setdefault("JAX_PLATFORMS", "axon")

import sys

if "/opt/trn_rl_repo" not in sys.path:
    sys.path.insert(0, "/opt/trn_rl_repo")

import numpy as np
import ml_dtypes

import concourse.bass as bass
import concourse.bacc as bacc
import concourse.mybir as mybir
import concourse.tile as tile
from concourse.bass_utils import run_bass_kernel_spmd
from concourse.masks import make_identity

P = 128
C = 768
CH = C // P            # 6 feature chunks
NH = 12
HD = 64
HH = 28                # image H = W
ROWS = 28              # attention rows per image
RG = 4                 # rows per slot
G = 7                  # slots per batch
W = RG * HH            # 112 queries per slot
SC = W + 1             # 113 keys per slot (cls + tokens)
S = G * SC             # 791 columns per batch
NB = 4                 # batches per core
TT = NB * S            # 3164 columns per core
NCORES = 8
B_TOTAL = 32
N_TOK = 1 + ROWS * HH  # 785
MPEN = 30.0            # mask penalty
HT = 4                 # heads per score tile
NT = NH // HT          # head tiles (3)
BANK = 512             # f32 elements per PSUM bank

F32 = mybir.dt.float32
BF16 = mybir.dt.bfloat16
BFNP = ml_dtypes.bfloat16


def _perm_valid():
    """original-token index for each of the S slot-layout columns + validity."""
    perm = np.zeros(S, np.int64)
    valid = np.ones(S, np.bool_)
    for g in range(G):
        perm[g * SC] = 0
        if g > 0:
            valid[g * SC] = False
        for j in range(W):
            r = RG * g + j // HH   # row index (original column w)
            i = j % HH             # position in row (original row h)
            perm[g * SC + 1 + j] = 1 + i * HH + r
    return perm, valid


def _consts():
    # multiplicative softmax mask, TRANSPOSED ([key, query]): exp(s + m - 30)
    # == exp(s) * M with M = 1 (cls key or same-row key) else e^-30
    m = np.full((SC, W), np.exp(-MPEN), np.float32)
    m[0, :] = 1.0
    for q in range(W):
        r = q // HH
        m[1 + r * HH : 1 + (r + 1) * HH, q] = 1.0
    # esel[h, c*128+p] = 1 iff h == 2c + (p >= 64): per-chunk selector that
    # broadcasts two heads' 1/Z rows onto the two 64-partition halves
    es = np.zeros((NH, CH * P), np.float32)
    for c in range(CH):
        es[2 * c, c * P : c * P + HD] = 1.0
        es[2 * c + 1, c * P + HD : (c + 1) * P] = 1.0
    return m.astype(BFNP), es.astype(BFNP)


def build_bass():
    nc = bacc.Bacc(None, target_bir_lowering=False, debug=True)

    x_t = nc.declare_dram_parameter("x_t", [C, TT], BF16, isOutput=False)
    w_qkv = nc.declare_dram_parameter("w_qkv", [C, 3 * C], BF16, isOutput=False)
    w_proj = nc.declare_dram_parameter("w_proj", [C, C], BF16, isOutput=False)
    b_pc = nc.declare_dram_parameter("b_pc", [P, CH], F32, isOutput=False)
    maskt = nc.declare_dram_parameter("maskt", [SC, W], BF16, isOutput=False)
    esel = nc.declare_dram_parameter("esel", [NH, CH * P], BF16, isOutput=False)
    out_t = nc.declare_dram_parameter("out_t", [C, TT], BF16, isOutput=True)

    NSPLIT = [(0, 512), (512, S - 512)]          # moving-dim tiling of S
    VSPLIT = [(0, 512), (512, C - 512)]          # moving-dim tiling of C (v cols)

    with tile.TileContext(nc) as tc:
        with (
            tc.tile_pool(name="const", bufs=1) as cpool,
            tc.tile_pool(name="xb", bufs=2) as xpool,
            tc.tile_pool(name="qk", bufs=2) as qkpool,
            tc.tile_pool(name="vt", bufs=2) as vpool,
            tc.tile_pool(name="yt", bufs=2) as ypool,
            tc.tile_pool(name="ob", bufs=2) as opool,
            tc.tile_pool(name="att", bufs=3) as apool,
            tc.tile_pool(name="attz", bufs=2) as zpool,
            tc.tile_pool(name="ps_mm", bufs=2, space="PSUM") as ps_mm,
            tc.tile_pool(name="ps_sc", bufs=1, space="PSUM") as ps_sc,
            tc.tile_pool(name="ps_z", bufs=1, space="PSUM") as ps_z,
            tc.tile_pool(name="ps_zb", bufs=1, space="PSUM") as ps_zb,
        ):
            # ---- constants ----
            wq_sb = cpool.tile([P, CH, 3 * C], BF16)
            nc.sync.dma_start(wq_sb[:], w_qkv.rearrange("(c p) o -> p c o", p=P))
            wp_sb = cpool.tile([P, CH, C], BF16)
            nc.sync.dma_start(wp_sb[:], w_proj.rearrange("(c p) o -> p c o", p=P))
            b_sb = cpool.tile([P, CH], F32)
            nc.sync.dma_start(b_sb[:], b_pc[:])
            mask_sb = cpool.tile([SC, W], BF16)
            nc.sync.dma_start(mask_sb[:], maskt[:])
            esel_sb = cpool.tile([NH, CH, P], BF16)
            nc.sync.dma_start(esel_sb[:], esel.rearrange("h (c p) -> h c p", p=P))
            ident = cpool.tile([W, W], F32)
            make_identity(nc, ident[:])
            onesp = cpool.tile([P, 1], BF16)
            nc.vector.memset(onesp[:], 1.0)
            onesrow = cpool.tile([1, P], F32)
            nc.vector.memset(onesrow[:], 1.0)

            def proj_batch(pb, yTp):
                cbp = pb * S
                ob = opool.tile([P, CH, S], BF16, tag="ob")
                for o in range(CH):
                    for (n0, nsz) in NSPLIT:
                        ps = ps_mm.tile([P, 512], F32, tag="mm")
                        for c in range(CH):
                            nc.tensor.matmul(
                                ps[:, :nsz],
                                lhsT=wp_sb[:, c, o * P : (o + 1) * P],
                                rhs=yTp[:, c, n0 : n0 + nsz],
                                start=(c == 0),
                                stop=(c == CH - 1),
                            )
                        nc.scalar.activation(
                            ob[:, o, n0 : n0 + nsz],
                            ps[:, :nsz],
                            mybir.ActivationFunctionType.Identity,
                            bias=b_sb[:, o : o + 1],
                        )
                nc.sync.dma_start(
                    out_t[:, cbp : cbp + S].rearrange("(c p) s -> p c s", p=P),
                    ob[:],
                )

            yT_prev = None
            for b in range(NB):
                cb = b * S
                xb = xpool.tile([P, CH, S], BF16, tag="xb")
                nc.sync.dma_start(
                    xb[:], x_t[:, cb : cb + S].rearrange("(c p) s -> p c s", p=P)
                )

                qT = qkpool.tile([P, CH, S], BF16, tag="qT")
                kT = qkpool.tile([P, CH, S], BF16, tag="kT")
                vtok = vpool.tile([P, G, C], BF16, tag="vtok")
                yT = ypool.tile([P, CH, S], BF16, tag="yT")
                # per-head Z for all slots: column NH*g+h (one 2KB bank)
                zall = ps_z.tile([W, G * NH], F32, tag="z")
                rzt = zpool.tile([W, G * NH], F32, tag="rzt", bufs=2)

                ut4s = {}
                rzbs = {}

                def qk_group(t, o):
                    dst = qT if o < CH else kT
                    oc = o % CH
                    for (n0, nsz) in NSPLIT:
                        ps = ps_mm.tile([P, 512], F32, tag="mm")
                        for c in range(CH):
                            nc.tensor.matmul(
                                ps[:, :nsz],
                                lhsT=wq_sb[:, c, o * P : (o + 1) * P],
                                rhs=xb[:, c, n0 : n0 + nsz],
                                start=(c == 0),
                                stop=(c == CH - 1),
                            )
                        nc.scalar.copy(dst[:, oc, n0 : n0 + nsz], ps[:, :nsz])

                def score_group(t, g):
                    # transposed scores: one matmul per head per PSUM bank
                    k0 = g * SC
                    sp = ps_sc.tile([SC, HT, BANK], F32, tag="sc")
                    for hh in range(HT):
                        h = HT * t + hh
                        c, hf = h // 2, (h % 2) * HD
                        nc.tensor.matmul(
                            sp[:, hh, :W],
                            lhsT=kT[hf : hf + HD, c, k0 : k0 + SC],
                            rhs=qT[hf : hf + HD, c, k0 + 1 : k0 + SC],
                            start=True,
                            stop=True,
                        )
                    ut4 = apool.tile([SC, HT, W], BF16, tag="ut", bufs=24)
                    nc.scalar.activation(
                        ut4[:], sp[:, :, :W], mybir.ActivationFunctionType.Exp
                    )
                    nc.vector.tensor_tensor(
                        ut4[:],
                        ut4[:],
                        mask_sb[:].rearrange("k q -> k () q").to_broadcast(
                            [SC, HT, W]
                        ),
                        mybir.AluOpType.mult,
                    )
                    ut4s[(g, t)] = ut4

                def v_split(g, v0, vsz):
                    ps = ps_mm.tile([P, 512], F32, tag="mm")
                    for c in range(CH):
                        nc.tensor.matmul(
                            ps[:SC, :vsz],
                            lhsT=xb[:, c, g * SC : (g + 1) * SC],
                            rhs=wq_sb[:, c, 2 * C + v0 : 2 * C + v0 + vsz],
                            start=(c == 0),
                            stop=(c == CH - 1),
                        )
                    nc.scalar.copy(vtok[:SC, g, v0 : v0 + vsz], ps[:SC, :vsz])

                def z_mms(g, t):
                    # per-head Z via tiny ones-matmuls into the shared bank
                    ut4 = ut4s[(g, t)]
                    for hh in range(HT):
                        col = NH * g + HT * t + hh
                        nc.tensor.matmul(
                            zall[:, col : col + 1],
                            lhsT=ut4[:, hh, :],
                            rhs=onesp[:SC, :],
                            start=True,
                            stop=True,
                        )

                def rz_chain(g):
                    # 1/Z then broadcast each head's row onto its
                    # 64-partition half via GpSimd
                    z_mms(g, 2)
                    nc.vector.reciprocal(
                        rzt[:, NH * g : NH * (g + 1)],
                        zall[:, NH * g : NH * (g + 1)],
                    )
                    rztp = ps_zb.tile([NH, W], F32, tag="zb")
                    nc.tensor.transpose(
                        rztp[:], rzt[:, NH * g : NH * (g + 1)], ident[:]
                    )
                    rzTs = zpool.tile([NH, W], BF16, tag="rzTs", bufs=3)
                    nc.vector.tensor_copy(rzTs[:], rztp[:])
                    rzb = zpool.tile([P, CH, W], BF16, tag="rzb", bufs=4)
                    for c in range(CH):
                        rzbp = ps_zb.tile([P, W], F32, tag="zb")
                        nc.tensor.matmul(
                            rzbp[:],
                            lhsT=esel_sb[:, c, :],
                            rhs=rzTs[:],
                            start=True,
                            stop=True,
                        )
                        nc.vector.tensor_copy(rzb[:, c, :], rzbp[:])
                    rzbs[g] = rzb

                def av_yt(g, cs):
                    k0 = g * SC
                    rzb = rzbs[g]
                    for c in cs:
                        t = c // 2
                        ut4 = ut4s[(g, t)]
                        yp = ps_mm.tile([P, 512], F32, tag="mm")
                        for hf in range(2):
                            h = 2 * c + hf
                            nc.tensor.matmul(
                                yp[hf * HD : (hf + 1) * HD, :W],
                                lhsT=vtok[:SC, g, h * HD : (h + 1) * HD],
                                rhs=ut4[:, h % HT, :],
                                start=True,
                                stop=True,
                            )
                        nc.vector.tensor_tensor(
                            yT[:, c, k0 + 1 : k0 + SC],
                            yp[:, :W],
                            rzb[:, c, :],
                            mybir.AluOpType.mult,
                        )

                # ---- phase A: qk pipelined with scores; deferred prev proj ----
                for o in (0, 1, CH, CH + 1):
                    qk_group(0, o)
                if yT_prev is not None:
                    proj_batch(b - 1, yT_prev)
                zq = []
                for t in (1, 2):
                    sg = 0
                    for o in (2 * t, 2 * t + 1, CH + 2 * t, CH + 2 * t + 1):
                        qk_group(t, o)
                        for _ in range(2):
                            if sg < G:
                                score_group(t - 1, sg)
                                zq.append((sg, t - 1))
                                sg += 1
                            if len(zq) >= 2:
                                z_mms(*zq.pop(0))
                while zq:
                    z_mms(*zq.pop(0))

                # cls logits + exp. qcls[:, c, j] holds the cls query of head
                # 2c+j on that head's 64 partitions and zeros elsewhere, so one
                # [128, SC]-contract matmul yields two heads' logits at once.
                qcls = zpool.tile([P, CH, 2], BF16, tag="qcls", bufs=2)
                nc.vector.memset(qcls[:], 0.0)
                nc.vector.tensor_copy(qcls[0:HD, :, 0], qT[0:HD, :, 0])
                nc.vector.tensor_copy(qcls[HD:P, :, 1], qT[HD:P, :, 0])
                ucts = []
                for g in range(G):
                    cp = ps_zb.tile([SC, NH], F32, tag="zb")
                    for c in range(CH):
                        nc.tensor.matmul(
                            cp[:, 2 * c : 2 * c + 2],
                            lhsT=kT[:, c, g * SC : (g + 1) * SC],
                            rhs=qcls[:, c, :],
                            start=True,
                            stop=True,
                            skip_group_check=True,
                        )
                    uct = apool.tile([SC, NH], BF16, tag="uct", bufs=9)
                    nc.scalar.activation(
                        uct[:], cp[:], mybir.ActivationFunctionType.Exp
                    )
                    if g > 0:
                        nc.vector.memset(uct[0:1, :], 0.0)
                    ucts.append(uct)

                # ---- v-projection stretch: v + t=2 scores + rz + AV, lagged ----
                for g in range(G):
                    v_split(g, *VSPLIT[0])
                    if g >= 1:
                        rz_chain(g - 1)
                    if g >= 2:
                        av_yt(g - 2, range(0, 3))
                    v_split(g, *VSPLIT[1])
                    if g >= 2:
                        av_yt(g - 2, range(3, CH))
                    score_group(2, g)
                rz_chain(G - 1)
                av_yt(G - 2, range(CH))
                av_yt(G - 1, range(CH))

                # ---- cls Z + normalize + AV ----
                zp = ps_zb.tile([1, NH], F32, tag="zb")
                for g in range(G):
                    nc.tensor.matmul(
                        zp[:],
                        lhsT=onesp[:SC, :],
                        rhs=ucts[g][:],
                        start=(g == 0),
                        stop=(g == G - 1),
                    )
                zcs = zpool.tile([1, NH], F32, tag="zcs")
                nc.vector.tensor_copy(zcs[:], zp[:])
                rzc = zpool.tile([1, NH], F32, tag="rzc")
                nc.vector.reciprocal(rzc[:], zcs[:])
                rzcb = ps_zb.tile([P, NH], F32, tag="zb")
                nc.tensor.matmul(
                    rzcb[:], lhsT=onesrow[:], rhs=rzc[:], start=True, stop=True
                )
                for g in range(G):
                    nc.vector.tensor_tensor(
                        ucts[g][:], ucts[g][:], rzcb[:SC, :], mybir.AluOpType.mult
                    )
                for c in range(CH):
                    yp = ps_mm.tile([P, 512], F32, tag="mm")
                    for g in range(G):
                        nc.tensor.matmul(
                            yp[:, 0:2],
                            lhsT=vtok[:SC, g, 2 * c * HD : (2 * c + 2) * HD],
                            rhs=ucts[g][:, 2 * c : 2 * c + 2],
                            start=(g == 0),
                            stop=(g == G - 1),
                        )
                    # diagonal halves: head 2c lives in col 0 rows 0:64,
                    # head 2c+1 in col 1 rows 64:128
                    nc.vector.tensor_copy(
                        yT[0:HD, c, 0 : S : SC],
                        yp[0:HD, 0:1].to_broadcast([HD, G]),
                    )
                    nc.vector.tensor_copy(
                        yT[HD:P, c, 0 : S : SC],
                        yp[HD:P, 1:2].to_broadcast([HD, G]),
                    )

                yT_prev = yT

            proj_batch(NB - 1, yT_prev)

    nc.compile()
    return nc


_NC_CACHE = None
_LAST_IN_MAPS = None


def kernel(x, w_qkv, w_proj, b_proj):
    global _NC_CACHE, _LAST_IN_MAPS
    x = np.asarray(x)
    w_qkv = np.asarray(w_qkv)
    w_proj = np.asarray(w_proj)
    b_proj = np.asarray(b_proj)

    perm, valid = _perm_valid()
    maskt, esel = _consts()

    wq = np.array(w_qkv, np.float32, copy=True)
    wq[:, :C] *= 1.0 / np.sqrt(HD)
    wq = wq.astype(BFNP)
    wp = w_proj.astype(BFNP)
    b_pc = np.ascontiguousarray(b_proj.astype(np.float32).reshape(CH, P).T)

    in_maps = []
    for core in range(NCORES):
        xs = x[core * NB : (core + 1) * NB]          # (NB, 785, C)
        xp = xs[:, perm, :]                          # (NB, S, C)
        x_T = np.ascontiguousarray(
            xp.transpose(2, 0, 1).reshape(C, TT)
        ).astype(BFNP)
        in_maps.append(
            {
                "x_t": x_T,
                "w_qkv": wq,
                "w_proj": wp,
                "b_pc": b_pc,
                "maskt": maskt,
                "esel": esel,
            }
        )

    if _NC_CACHE is None:
        _NC_CACHE = build_bass()
    nc = _NC_CACHE

    _LAST_IN_MAPS = in_maps

    res = run_bass_kernel_spmd(nc, in_maps, core_ids=list(range(NCORES)))

    out = np.zeros((B_TOTAL, N_TOK, C), np.float32)
    vperm = perm[valid]
    for core in range(NCORES):
        o_t = np.asarray(res.results[core]["out_t"], dtype=np.float32)  # (C, TT)
        op = o_t.reshape(C, NB, S).transpose(1, 2, 0)  # (NB, S, C)
        out[core * NB : (core + 1) * NB][:, vperm, :] = op[:, valid, :]
    return out


if __name__ == "__main__":
    rng = np.random.default_rng(0)
    x = rng.standard_normal((B_TOTAL, N_TOK, C)).astype(np.float32)
    w_qkv = (rng.standard_normal((C, 3 * C)) * 0.02).astype(np.float32)
    w_proj = (rng.standard_normal((C, C)) * 0.02).astype(np.float32)
    b_proj = np.zeros((C,), np.float32)
    y = kernel(x=x, w_qkv=w_qkv, w_proj=w_proj, b_proj=b_proj)
    print(y.shape, y.dtype)


# revision 25
# speedup vs baseline: 1.1455x; 1.0250x over previous
"""AxialAttention (vertical, cls token, full cls attention) Trainium2 kernel.

Strategy: data-parallel over batch (32 batches -> 8 cores x 4 batches).
Per core everything is done in one fused Bass/Tile program:
  qkv projection -> per-row axial attention (+ full cls attention) -> out proj.

Host-side layout (per batch element):
  tokens are regrouped into 7 "slots" of 113 columns: [cls, 4 rows x 28 tok].
  Everything on-chip is feature-major (C on partitions): x_T (768, 791*4).

Row attention computes scores TRANSPOSED (keys on partitions) so no PE
transpose of the attention weights is needed:
  scoresT[k, q] = k_slot.T @ q_slot per head, 4 heads per PSUM tile with one
  bank per head (HW requires one matmul group per 2KB PSUM bank), one exp per
  4-head tile straight to SBUF, one multiplicative-mask multiply (exp(s+m-30)
  == exp(s)*M with M=1 valid / e^-30 invalid), per-head Z via tiny ones-matmuls
  into a shared [112, 84] PSUM tile, reciprocal, and a per-(slot, chunk)
  PE-broadcast of 1/Z (eye2-matmul) applied at the yT writeback
  (tensor_tensor multiply instead of a copy).
cls attention result is broadcast into all 7 dup-cls columns of yT so no
yT memset is needed.

Emission is software-pipelined per batch so each engine gets long runs of
independent work: qk chunk-pairs interleaved with score tiles, the previous
batch's projection as dense cover, then a v-projection stretch carrying the
t=2 scores, the 1/Z chains and the AV+writeback with 1-2 slot lag.
"""

import os

os.environ.setdefault("JAX_PLATFORMS", "axon")

import sys

if "/opt/trn_rl_repo" not in sys.path:
    sys.path.insert(0, "/opt/trn_rl_repo")

import numpy as np
import ml_dtypes

import concourse.bass as bass
import concourse.bacc as bacc
import concourse.mybir as mybir
import concourse.tile as tile
from concourse.bass_utils import run_bass_kernel_spmd
from concourse.masks import make_identity

P = 128
C = 768
CH = C // P            # 6 feature chunks
NH = 12
HD = 64
HH = 28                # image H = W
ROWS = 28              # attention rows per image
RG = 4                 # rows per slot
G = 7                  # slots per batch
W = RG * HH            # 112 queries per slot
SC = W + 1             # 113 keys per slot (cls + tokens)
S = G * SC             # 791 columns per batch
NB = 4                 # batches per core
TT = NB * S            # 3164 columns per core
NCORES = 8
B_TOTAL = 32
N_TOK = 1 + ROWS * HH  # 785
MPEN = 30.0            # mask penalty
HT = 4                 # heads per score tile
NT = NH // HT          # head tiles (3)
BANK = 512             # f32 elements per PSUM bank

F32 = mybir.dt.float32
BF16 = mybir.dt.bfloat16
BFNP = ml_dtypes.bfloat16


def _perm_valid():
    """original-token index for each of the S slot-layout columns + validity."""
    perm = np.zeros(S, np.int64)
    valid = np.ones(S, np.bool_)
    for g in range(G):
        perm[g * SC] = 0
        if g > 0:
            valid[g * SC] = False
        for j in range(W):
            r = RG * g + j // HH   # row index (original column w)
            i = j % HH             # position in row (original row h)
            perm[g * SC + 1 + j] = 1 + i * HH + r
    return perm, valid


def _consts():
    # multiplicative softmax mask, TRANSPOSED ([key, query]): exp(s + m - 30)
    # == exp(s) * M with M = 1 (cls key or same-row key) else e^-30
    m = np.full((SC, W), np.exp(-MPEN), np.float32)
    m[0, :] = 1.0
    for q in range(W):
        r = q // HH
        m[1 + r * HH : 1 + (r + 1) * HH, q] = 1.0
    # esel[h, c*128+p] = 1 iff h == 2c + (p >= 64): per-chunk selector that
    # broadcasts two heads' 1/Z rows onto the two 64-partition halves
    es = np.zeros((NH, CH * P), np.float32)
    for c in range(CH):
        es[2 * c, c * P : c * P + HD] = 1.0
        es[2 * c + 1, c * P + HD : (c + 1) * P] = 1.0
    return m.astype(BFNP), es.astype(BFNP)


def build_bass():
    nc = bacc.Bacc(None, target_bir_lowering=False, debug=True)

    x_t = nc.declare_dram_parameter("x_t", [C, TT], BF16, isOutput=False)
    w_qkv = nc.declare_dram_parameter("w_qkv", [C, 3 * C], BF16, isOutput=False)
    w_proj = nc.declare_dram_parameter("w_proj", [C, C], BF16, isOutput=False)
    b_pc = nc.declare_dram_parameter("b_pc", [P, CH], F32, isOutput=False)
    maskt = nc.declare_dram_parameter("maskt", [SC, W], BF16, isOutput=False)
    esel = nc.declare_dram_parameter("esel", [NH, CH * P], BF16, isOutput=False)
    out_t = nc.declare_dram_parameter("out_t", [C, TT], BF16, isOutput=True)

    NSPLIT = [(0, 512), (512, S - 512)]          # moving-dim tiling of S
    VSPLIT = [(0, 512), (512, C - 512)]          # moving-dim tiling of C (v cols)

    with tile.TileContext(nc) as tc:
        with (
            tc.tile_pool(name="const", bufs=1) as cpool,
            tc.tile_pool(name="xb", bufs=2) as xpool,
            tc.tile_pool(name="qk", bufs=2) as qkpool,
            tc.tile_pool(name="vt", bufs=2) as vpool,
            tc.tile_pool(name="yt", bufs=2) as ypool,
            tc.tile_pool(name="ob", bufs=2) as opool,
            tc.tile_pool(name="att", bufs=3) as apool,
            tc.tile_pool(name="attz", bufs=2) as zpool,
            tc.tile_pool(name="ps_mm", bufs=2, space="PSUM") as ps_mm,
            tc.tile_pool(name="ps_sc", bufs=1, space="PSUM") as ps_sc,
            tc.tile_pool(name="ps_z", bufs=1, space="PSUM") as ps_z,
            tc.tile_pool(name="ps_zb", bufs=1, space="PSUM") as ps_zb,
        ):
            # ---- constants ----
            wq_sb = cpool.tile([P, CH, 3 * C], BF16)
            nc.sync.dma_start(wq_sb[:], w_qkv.rearrange("(c p) o -> p c o", p=P))
            wp_sb = cpool.tile([P, CH, C], BF16)
            nc.sync.dma_start(wp_sb[:], w_proj.rearrange("(c p) o -> p c o", p=P))
            b_sb = cpool.tile([P, CH], F32)
            nc.sync.dma_start(b_sb[:], b_pc[:])
            mask_sb = cpool.tile([SC, W], BF16)
            nc.sync.dma_start(mask_sb[:], maskt[:])
            esel_sb = cpool.tile([NH, CH, P], BF16)
            nc.sync.dma_start(esel_sb[:], esel.rearrange("h (c p) -> h c p", p=P))
            ident = cpool.tile([W, W], F32)
            make_identity(nc, ident[:])
            onesp = cpool.tile([P, 1], BF16)
            nc.vector.memset(onesp[:], 1.0)
            onesrow = cpool.tile([1, P], F32)
            nc.vector.memset(onesrow[:], 1.0)

            def proj_unit(yTp, ob, o, n0, nsz):
                ps = ps_mm.tile([P, 512], F32, tag="mm")
                for c in range(CH):
                    nc.tensor.matmul(
                        ps[:, :nsz],
                        lhsT=wp_sb[:, c, o * P : (o + 1) * P],
                        rhs=yTp[:, c, n0 : n0 + nsz],
                        start=(c == 0),
                        stop=(c == CH - 1),
                    )
                nc.scalar.activation(
                    ob[:, o, n0 : n0 + nsz],
                    ps[:, :nsz],
                    mybir.ActivationFunctionType.Identity,
                    bias=b_sb[:, o : o + 1],
                )

            def proj_batch(pb, yTp):
                ob = opool.tile([P, CH, S], BF16, tag="ob")
                for o in range(CH):
                    for (n0, nsz) in NSPLIT:
                        proj_unit(yTp, ob, o, n0, nsz)
                nc.sync.dma_start(
                    out_t[:, pb * S : (pb + 1) * S].rearrange(
                        "(c p) s -> p c s", p=P
                    ),
                    ob[:],
                )

            yT_prev = None
            for b in range(NB):
                cb = b * S
                xb = xpool.tile([P, CH, S], BF16, tag="xb")
                nc.sync.dma_start(
                    xb[:], x_t[:, cb : cb + S].rearrange("(c p) s -> p c s", p=P)
                )

                qT = qkpool.tile([P, CH, S], BF16, tag="qT")
                kT = qkpool.tile([P, CH, S], BF16, tag="kT")
                vtok = vpool.tile([P, G, C], BF16, tag="vtok")
                yT = ypool.tile([P, CH, S], BF16, tag="yT")
                # per-head Z for all slots: column NH*g+h (one 2KB bank)
                zall = ps_z.tile([W, G * NH], F32, tag="z")
                rzt = zpool.tile([W, G * NH], F32, tag="rzt", bufs=2)

                ut4s = {}
                rzbs = {}

                def qk_group(t, o):
                    dst = qT if o < CH else kT
                    oc = o % CH
                    for (n0, nsz) in NSPLIT:
                        ps = ps_mm.tile([P, 512], F32, tag="mm")
                        for c in range(CH):
                            nc.tensor.matmul(
                                ps[:, :nsz],
                                lhsT=wq_sb[:, c, o * P : (o + 1) * P],
                                rhs=xb[:, c, n0 : n0 + nsz],
                                start=(c == 0),
                                stop=(c == CH - 1),
                            )
                        nc.scalar.copy(dst[:, oc, n0 : n0 + nsz], ps[:, :nsz])

                def score_group(t, g):
                    # transposed scores: one matmul per head per PSUM bank
                    k0 = g * SC
                    sp = ps_sc.tile([SC, HT, BANK], F32, tag="sc")
                    for hh in range(HT):
                        h = HT * t + hh
                        c, hf = h // 2, (h % 2) * HD
                        nc.tensor.matmul(
                            sp[:, hh, :W],
                            lhsT=kT[hf : hf + HD, c, k0 : k0 + SC],
                            rhs=qT[hf : hf + HD, c, k0 + 1 : k0 + SC],
                            start=True,
                            stop=True,
                        )
                    ut4 = apool.tile([SC, HT, W], BF16, tag="ut", bufs=24)
                    nc.scalar.activation(
                        ut4[:], sp[:, :, :W], mybir.ActivationFunctionType.Exp
                    )
                    nc.vector.tensor_tensor(
                        ut4[:],
                        ut4[:],
                        mask_sb[:].rearrange("k q -> k () q").to_broadcast(
                            [SC, HT, W]
                        ),
                        mybir.AluOpType.mult,
                    )
                    ut4s[(g, t)] = ut4

                def v_split(g, v0, vsz):
                    ps = ps_mm.tile([P, 512], F32, tag="mm")
                    for c in range(CH):
                        nc.tensor.matmul(
                            ps[:SC, :vsz],
                            lhsT=xb[:, c, g * SC : (g + 1) * SC],
                            rhs=wq_sb[:, c, 2 * C + v0 : 2 * C + v0 + vsz],
                            start=(c == 0),
                            stop=(c == CH - 1),
                        )
                    nc.scalar.copy(vtok[:SC, g, v0 : v0 + vsz], ps[:SC, :vsz])

                def z_mms(g, t):
                    # per-head Z via tiny ones-matmuls into the shared bank
                    ut4 = ut4s[(g, t)]
                    for hh in range(HT):
                        col = NH * g + HT * t + hh
                        nc.tensor.matmul(
                            zall[:, col : col + 1],
                            lhsT=ut4[:, hh, :],
                            rhs=onesp[:SC, :],
                            start=True,
                            stop=True,
                        )

                def rz_chain(g):
                    # 1/Z then broadcast each head's row onto its
                    # 64-partition half via GpSimd
                    z_mms(g, 2)
                    nc.vector.reciprocal(
                        rzt[:, NH * g : NH * (g + 1)],
                        zall[:, NH * g : NH * (g + 1)],
                    )
                    rztp = ps_zb.tile([NH, W], F32, tag="zb")
                    nc.tensor.transpose(
                        rztp[:], rzt[:, NH * g : NH * (g + 1)], ident[:]
                    )
                    rzTs = zpool.tile([NH, W], BF16, tag="rzTs", bufs=3)
                    nc.vector.tensor_copy(rzTs[:], rztp[:])
                    rzb = zpool.tile([P, CH, W], BF16, tag="rzb", bufs=4)
                    for c in range(CH):
                        rzbp = ps_zb.tile([P, W], F32, tag="zb")
                        nc.tensor.matmul(
                            rzbp[:],
                            lhsT=esel_sb[:, c, :],
                            rhs=rzTs[:],
                            start=True,
                            stop=True,
                        )
                        nc.vector.tensor_copy(rzb[:, c, :], rzbp[:])
                    rzbs[g] = rzb

                def av_yt(g, cs):
                    k0 = g * SC
                    rzb = rzbs[g]
                    for c in cs:
                        t = c // 2
                        ut4 = ut4s[(g, t)]
                        yp = ps_mm.tile([P, 512], F32, tag="mm")
                        for hf in range(2):
                            h = 2 * c + hf
                            nc.tensor.matmul(
                                yp[hf * HD : (hf + 1) * HD, :W],
                                lhsT=vtok[:SC, g, h * HD : (h + 1) * HD],
                                rhs=ut4[:, h % HT, :],
                                start=True,
                                stop=True,
                            )
                        nc.vector.tensor_tensor(
                            yT[:, c, k0 + 1 : k0 + SC],
                            yp[:, :W],
                            rzb[:, c, :],
                            mybir.AluOpType.mult,
                        )

                # ---- phase A: qk pipelined with scores; deferred prev proj ----
                for o in (0, 1, CH, CH + 1):
                    qk_group(0, o)
                ob_prev = None
                if yT_prev is not None:
                    ob_prev = opool.tile([P, CH, S], BF16, tag="ob")
                    proj_units = [
                        (o, n0, nsz) for o in range(CH) for (n0, nsz) in NSPLIT
                    ]
                zq = []
                for t in (1, 2):
                    sg = 0
                    for o in (2 * t, 2 * t + 1, CH + 2 * t, CH + 2 * t + 1):
                        qk_group(t, o)
                        for _ in range(2):
                            if sg < G:
                                score_group(t - 1, sg)
                                zq.append((sg, t - 1))
                                sg += 1
                            if len(zq) >= 2:
                                z_mms(*zq.pop(0))
                while zq:
                    z_mms(*zq.pop(0))

                # cls logits + exp. qcls[:, c, j] holds the cls query of head
                # 2c+j on that head's 64 partitions and zeros elsewhere, so one
                # [128, SC]-contract matmul yields two heads' logits at once.
                qcls = zpool.tile([P, CH, 2], BF16, tag="qcls", bufs=2)
                nc.vector.memset(qcls[:], 0.0)
                nc.vector.tensor_copy(qcls[0:HD, :, 0], qT[0:HD, :, 0])
                nc.vector.tensor_copy(qcls[HD:P, :, 1], qT[HD:P, :, 0])
                ucts = []
                for g in range(G):
                    cp = ps_zb.tile([SC, NH], F32, tag="zb")
                    for c in range(CH):
                        nc.tensor.matmul(
                            cp[:, 2 * c : 2 * c + 2],
                            lhsT=kT[:, c, g * SC : (g + 1) * SC],
                            rhs=qcls[:, c, :],
                            start=True,
                            stop=True,
                            skip_group_check=True,
                        )
                    uct = apool.tile([SC, NH], BF16, tag="uct", bufs=9)
                    nc.scalar.activation(
                        uct[:], cp[:], mybir.ActivationFunctionType.Exp
                    )
                    if g > 0:
                        nc.vector.memset(uct[0:1, :], 0.0)
                    ucts.append(uct)

                # ---- v-projection stretch: v + t=2 scores + rz + AV, lagged ----
                for g in range(G):
                    v_split(g, *VSPLIT[0])
                    if ob_prev is not None and proj_units:
                        proj_unit(yT_prev, ob_prev, *proj_units.pop(0))
                    if g >= 1:
                        rz_chain(g - 1)
                    if g >= 2:
                        av_yt(g - 2, range(0, 3))
                    v_split(g, *VSPLIT[1])
                    if ob_prev is not None and proj_units:
                        proj_unit(yT_prev, ob_prev, *proj_units.pop(0))
                    if g >= 2:
                        av_yt(g - 2, range(3, CH))
                    score_group(2, g)
                rz_chain(G - 1)
                av_yt(G - 2, range(CH))
                if ob_prev is not None:
                    while proj_units:
                        proj_unit(yT_prev, ob_prev, *proj_units.pop(0))
                    nc.sync.dma_start(
                        out_t[:, (b - 1) * S : b * S].rearrange(
                            "(c p) s -> p c s", p=P
                        ),
                        ob_prev[:],
                    )
                av_yt(G - 1, range(CH))

                # ---- cls Z + normalize + AV ----
                zp = ps_zb.tile([1, NH], F32, tag="zb")
                for g in range(G):
                    nc.tensor.matmul(
                        zp[:],
                        lhsT=onesp[:SC, :],
                        rhs=ucts[g][:],
                        start=(g == 0),
                        stop=(g == G - 1),
                    )
                zcs = zpool.tile([1, NH], F32, tag="zcs")
                nc.vector.tensor_copy(zcs[:], zp[:])
                rzc = zpool.tile([1, NH], F32, tag="rzc")
                nc.vector.reciprocal(rzc[:], zcs[:])
                rzcb = ps_zb.tile([P, NH], F32, tag="zb")
                nc.tensor.matmul(
                    rzcb[:], lhsT=onesrow[:], rhs=rzc[:], start=True, stop=True
                )
                for g in range(G):
                    nc.vector.tensor_tensor(
                        ucts[g][:], ucts[g][:], rzcb[:SC, :], mybir.AluOpType.mult
                    )
                for c in range(CH):
                    yp = ps_mm.tile([P, 512], F32, tag="mm")
                    for g in range(G):
                        nc.tensor.matmul(
                            yp[:, 0:2],
                            lhsT=vtok[:SC, g, 2 * c * HD : (2 * c + 2) * HD],
                            rhs=ucts[g][:, 2 * c : 2 * c + 2],
                            start=(g == 0),
                            stop=(g == G - 1),
                        )
                    # diagonal halves: head 2c lives in col 0 rows 0:64,
                    # head 2c+1 in col 1 rows 64:128
                    nc.vector.tensor_copy(
                        yT[0:HD, c, 0 : S : SC],
                        yp[0:HD, 0:1].to_broadcast([HD, G]),
                    )
                    nc.vector.tensor_copy(
                        yT[HD:P, c, 0 : S : SC],
                        yp[HD:P, 1:2].to_broadcast([HD, G]),
                    )

                yT_prev = yT

            proj_batch(NB - 1, yT_prev)

    nc.compile()
    return nc


_NC_CACHE = None
_LAST_IN_MAPS = None


def kernel(x, w_qkv, w_proj, b_proj):
    global _NC_CACHE, _LAST_IN_MAPS
    x = np.asarray(x)
    w_qkv = np.asarray(w_qkv)
    w_proj = np.asarray(w_proj)
    b_proj = np.asarray(b_proj)

    perm, valid = _perm_valid()
    maskt, esel = _consts()

    wq = np.array(w_qkv, np.float32, copy=True)
    wq[:, :C] *= 1.0 / np.sqrt(HD)
    wq = wq.astype(BFNP)
    wp = w_proj.astype(BFNP)
    b_pc = np.ascontiguousarray(b_proj.astype(np.float32).reshape(CH, P).T)

    in_maps = []
    for core in range(NCORES):
        xs = x[core * NB : (core + 1) * NB]          # (NB, 785, C)
        xp = xs[:, perm, :]                          # (NB, S, C)
        x_T = np.ascontiguousarray(
            xp.transpose(2, 0, 1).reshape(C, TT)
        ).astype(BFNP)
        in_maps.append(
            {
                "x_t": x_T,
                "w_qkv": wq,
                "w_proj": wp,
                "b_pc": b_pc,
                "maskt": maskt,
                "esel": esel,
            }
        )

    if _NC_CACHE is None:
        _NC_CACHE = build_bass()
    nc = _NC_CACHE

    _LAST_IN_MAPS = in_maps

    res = run_bass_kernel_spmd(nc, in_maps, core_ids=list(range(NCORES)))

    out = np.zeros((B_TOTAL, N_TOK, C), np.float32)
    vperm = perm[valid]
    for core in range(NCORES):
        o_t = np.asarray(res.results[core]["out_t"], dtype=np.float32)  # (C, TT)
        op = o_t.reshape(C, NB, S).transpose(1, 2, 0)  # (NB, S, C)
        out[core * NB : (core + 1) * NB][:, vperm, :] = op[:, valid, :]
    return out


if __name__ == "__main__":
    rng = np.random.default_rng(0)
    x = rng.standard_normal((B_TOTAL, N_TOK, C)).astype(np.float32)
    w_qkv = (rng.standard_normal((C, 3 * C)) * 0.02).astype(np.float32)
    w_proj = (rng.standard_normal((C, C)) * 0.02).astype(np.float32)
    b_proj = np.zeros((C,), np.float32)
    y = kernel(x=x, w_qkv=w_qkv, w_proj=w_proj, b_proj=b_proj)
    print(y.shape, y.dtype)


# revision 29
# speedup vs baseline: 1.1619x; 1.0143x over previous
"""AxialAttention (vertical, cls token, full cls attention) Trainium2 kernel.

Strategy: data-parallel over batch (32 batches -> 8 cores x 4 batches).
Per core everything is done in one fused Bass/Tile program:
  qkv projection -> per-row axial attention (+ full cls attention) -> out proj.

Host-side layout (per batch element):
  tokens are regrouped into 7 "slots" of 113 columns: [cls, 4 rows x 28 tok].
  Everything on-chip is feature-major (C on partitions): x_T (768, 791*4).

Row attention computes scores TRANSPOSED (keys on partitions) so no PE
transpose of the attention weights is needed:
  scoresT[k, q] = k_slot.T @ q_slot per head, 4 heads per PSUM tile with one
  bank per head (HW requires one matmul group per 2KB PSUM bank), one exp per
  4-head tile straight to SBUF, one multiplicative-mask multiply (exp(s+m-30)
  == exp(s)*M with M=1 valid / e^-30 invalid), per-head Z via tiny ones-matmuls
  into a shared [112, 84] PSUM tile, reciprocal, and a per-(slot, chunk)
  PE-broadcast of 1/Z (eye2-matmul) applied at the yT writeback
  (tensor_tensor multiply instead of a copy).
cls attention result is broadcast into all 7 dup-cls columns of yT so no
yT memset is needed.

Emission is software-pipelined per batch so each engine gets long runs of
independent work: qk chunk-pairs interleaved with score tiles, the previous
batch's projection as dense cover, then a v-projection stretch carrying the
t=2 scores, the 1/Z chains and the AV+writeback with 1-2 slot lag.
"""

import os

os.environ.setdefault("JAX_PLATFORMS", "axon")

import sys

if "/opt/trn_rl_repo" not in sys.path:
    sys.path.insert(0, "/opt/trn_rl_repo")

import numpy as np
import ml_dtypes

import concourse.bass as bass
import concourse.bacc as bacc
import concourse.mybir as mybir
import concourse.tile as tile
from concourse.bass_utils import run_bass_kernel_spmd
from concourse.masks import make_identity

P = 128
C = 768
CH = C // P            # 6 feature chunks
NH = 12
HD = 64
HH = 28                # image H = W
ROWS = 28              # attention rows per image
RG = 4                 # rows per slot
G = 7                  # slots per batch
W = RG * HH            # 112 queries per slot
SC = W + 1             # 113 keys per slot (cls + tokens)
S = G * SC             # 791 columns per batch
NB = 4                 # batches per core
TT = NB * S            # 3164 columns per core
NCORES = 8
B_TOTAL = 32
N_TOK = 1 + ROWS * HH  # 785
MPEN = 30.0            # mask penalty
HT = 4                 # heads per score tile
NT = NH // HT          # head tiles (3)
BANK = 512             # f32 elements per PSUM bank

F32 = mybir.dt.float32
BF16 = mybir.dt.bfloat16
BFNP = ml_dtypes.bfloat16


def _perm_valid():
    """original-token index for each of the S slot-layout columns + validity."""
    perm = np.zeros(S, np.int64)
    valid = np.ones(S, np.bool_)
    for g in range(G):
        perm[g * SC] = 0
        if g > 0:
            valid[g * SC] = False
        for j in range(W):
            r = RG * g + j // HH   # row index (original column w)
            i = j % HH             # position in row (original row h)
            perm[g * SC + 1 + j] = 1 + i * HH + r
    return perm, valid


def _consts():
    # multiplicative softmax mask, TRANSPOSED ([key, query]): exp(s + m - 30)
    # == exp(s) * M with M = 1 (cls key or same-row key) else e^-30
    m = np.full((SC, W), np.exp(-MPEN), np.float32)
    m[0, :] = 1.0
    for q in range(W):
        r = q // HH
        m[1 + r * HH : 1 + (r + 1) * HH, q] = 1.0
    # esel[h, c*128+p] = 1 iff h == 2c + (p >= 64): per-chunk selector that
    # broadcasts two heads' 1/Z rows onto the two 64-partition halves
    es = np.zeros((NH, CH * P), np.float32)
    for c in range(CH):
        es[2 * c, c * P : c * P + HD] = 1.0
        es[2 * c + 1, c * P + HD : (c + 1) * P] = 1.0
    return m.astype(BFNP), es.astype(BFNP)


def build_bass():
    nc = bacc.Bacc(None, target_bir_lowering=False, debug=True)

    x_t = nc.declare_dram_parameter("x_t", [C, TT], BF16, isOutput=False)
    w_qkv = nc.declare_dram_parameter("w_qkv", [C, 3 * C], BF16, isOutput=False)
    w_proj = nc.declare_dram_parameter("w_proj", [C, C], BF16, isOutput=False)
    b_pc = nc.declare_dram_parameter("b_pc", [P, CH], F32, isOutput=False)
    maskt = nc.declare_dram_parameter("maskt", [SC, W], BF16, isOutput=False)
    esel = nc.declare_dram_parameter("esel", [NH, CH * P], BF16, isOutput=False)
    out_t = nc.declare_dram_parameter("out_t", [C, TT], BF16, isOutput=True)

    NSPLIT = [(0, 512), (512, S - 512)]          # moving-dim tiling of S
    VSPLIT = [(0, 512), (512, C - 512)]          # moving-dim tiling of C (v cols)

    with tile.TileContext(nc) as tc:
        with (
            tc.tile_pool(name="const", bufs=1) as cpool,
            tc.tile_pool(name="xb", bufs=2) as xpool,
            tc.tile_pool(name="qk", bufs=2) as qkpool,
            tc.tile_pool(name="vt", bufs=2) as vpool,
            tc.tile_pool(name="yt", bufs=2) as ypool,
            tc.tile_pool(name="ob", bufs=2) as opool,
            tc.tile_pool(name="att", bufs=3) as apool,
            tc.tile_pool(name="attz", bufs=2) as zpool,
            tc.tile_pool(name="ps_mm", bufs=2, space="PSUM") as ps_mm,
            tc.tile_pool(name="ps_sc", bufs=1, space="PSUM") as ps_sc,
            tc.tile_pool(name="ps_z", bufs=1, space="PSUM") as ps_z,
            tc.tile_pool(name="ps_zb", bufs=1, space="PSUM") as ps_zb,
        ):
            # ---- constants ----
            wq_sb = cpool.tile([P, CH, 3 * C], BF16)
            nc.sync.dma_start(wq_sb[:], w_qkv.rearrange("(c p) o -> p c o", p=P))
            wp_sb = cpool.tile([P, CH, C], BF16)
            nc.sync.dma_start(wp_sb[:], w_proj.rearrange("(c p) o -> p c o", p=P))
            b_sb = cpool.tile([P, CH], F32)
            nc.sync.dma_start(b_sb[:], b_pc[:])
            mask_sb = cpool.tile([SC, W], BF16)
            nc.sync.dma_start(mask_sb[:], maskt[:])
            esel_sb = cpool.tile([NH, CH, P], BF16)
            nc.sync.dma_start(esel_sb[:], esel.rearrange("h (c p) -> h c p", p=P))
            ident = cpool.tile([W, W], F32)
            make_identity(nc, ident[:])
            onesp = cpool.tile([P, 1], BF16)
            nc.vector.memset(onesp[:], 1.0)
            onesrow = cpool.tile([1, P], F32)
            nc.vector.memset(onesrow[:], 1.0)

            def proj_unit(yTp, ob, o, n0, nsz):
                ps = ps_mm.tile([P, 512], F32, tag="mm")
                for c in range(CH):
                    nc.tensor.matmul(
                        ps[:, :nsz],
                        lhsT=wp_sb[:, c, o * P : (o + 1) * P],
                        rhs=yTp[:, c, n0 : n0 + nsz],
                        start=(c == 0),
                        stop=(c == CH - 1),
                    )
                nc.scalar.activation(
                    ob[:, o, n0 : n0 + nsz],
                    ps[:, :nsz],
                    mybir.ActivationFunctionType.Identity,
                    bias=b_sb[:, o : o + 1],
                )

            def proj_batch(pb, yTp):
                ob = opool.tile([P, CH, S], BF16, tag="ob")
                for o in range(CH):
                    for (n0, nsz) in NSPLIT:
                        proj_unit(yTp, ob, o, n0, nsz)
                nc.sync.dma_start(
                    out_t[:, pb * S : (pb + 1) * S].rearrange(
                        "(c p) s -> p c s", p=P
                    ),
                    ob[:],
                )

            yT_prev = None
            for b in range(NB):
                cb = b * S
                xb = xpool.tile([P, CH, S], BF16, tag="xb")
                nc.sync.dma_start(
                    xb[:], x_t[:, cb : cb + S].rearrange("(c p) s -> p c s", p=P)
                )

                qT = qkpool.tile([P, CH, S], BF16, tag="qT")
                kT = qkpool.tile([P, CH, S], BF16, tag="kT")
                vtok = vpool.tile([P, G, C], BF16, tag="vtok")
                yT = ypool.tile([P, CH, S], BF16, tag="yT")
                # per-head Z for all slots: column NH*g+h (one 2KB bank)
                zall = ps_z.tile([W, G * NH], F32, tag="z")
                rzt = zpool.tile([W, G * NH], F32, tag="rzt", bufs=2)

                ut4s = {}
                rzbs = {}

                def qk_group(t, o):
                    dst = qT if o < CH else kT
                    oc = o % CH
                    for (n0, nsz) in NSPLIT:
                        ps = ps_mm.tile([P, 512], F32, tag="mm")
                        for c in range(CH):
                            nc.tensor.matmul(
                                ps[:, :nsz],
                                lhsT=wq_sb[:, c, o * P : (o + 1) * P],
                                rhs=xb[:, c, n0 : n0 + nsz],
                                start=(c == 0),
                                stop=(c == CH - 1),
                            )
                        nc.scalar.copy(dst[:, oc, n0 : n0 + nsz], ps[:, :nsz])

                def score_group(t, g):
                    # transposed scores: one matmul per head per PSUM bank
                    k0 = g * SC
                    sp = ps_sc.tile([SC, HT, BANK], F32, tag="sc")
                    for hh in range(HT):
                        h = HT * t + hh
                        c, hf = h // 2, (h % 2) * HD
                        nc.tensor.matmul(
                            sp[:, hh, :W],
                            lhsT=kT[hf : hf + HD, c, k0 : k0 + SC],
                            rhs=qT[hf : hf + HD, c, k0 + 1 : k0 + SC],
                            start=True,
                            stop=True,
                        )
                    ut4 = apool.tile([SC, HT, W], BF16, tag="ut", bufs=24)
                    nc.scalar.activation(
                        ut4[:], sp[:, :, :W], mybir.ActivationFunctionType.Exp
                    )
                    nc.vector.tensor_tensor(
                        ut4[:],
                        ut4[:],
                        mask_sb[:].rearrange("k q -> k () q").to_broadcast(
                            [SC, HT, W]
                        ),
                        mybir.AluOpType.mult,
                    )
                    ut4s[(g, t)] = ut4

                def v_split(g, v0, vsz):
                    ps = ps_mm.tile([P, 512], F32, tag="mm")
                    for c in range(CH):
                        nc.tensor.matmul(
                            ps[:SC, :vsz],
                            lhsT=xb[:, c, g * SC : (g + 1) * SC],
                            rhs=wq_sb[:, c, 2 * C + v0 : 2 * C + v0 + vsz],
                            start=(c == 0),
                            stop=(c == CH - 1),
                        )
                    nc.scalar.copy(vtok[:SC, g, v0 : v0 + vsz], ps[:SC, :vsz])

                def z_mms(g, t):
                    # per-head Z via tiny ones-matmuls into the shared bank
                    ut4 = ut4s[(g, t)]
                    for hh in range(HT):
                        col = NH * g + HT * t + hh
                        nc.tensor.matmul(
                            zall[:, col : col + 1],
                            lhsT=ut4[:, hh, :],
                            rhs=onesp[:SC, :],
                            start=True,
                            stop=True,
                        )

                def rz_chain(g):
                    # 1/Z then broadcast each head's row onto its
                    # 64-partition half via GpSimd
                    z_mms(g, 2)
                    nc.vector.reciprocal(
                        rzt[:, NH * g : NH * (g + 1)],
                        zall[:, NH * g : NH * (g + 1)],
                    )
                    rztp = ps_zb.tile([NH, W], F32, tag="zb")
                    nc.tensor.transpose(
                        rztp[:], rzt[:, NH * g : NH * (g + 1)], ident[:]
                    )
                    rzTs = zpool.tile([NH, W], BF16, tag="rzTs", bufs=3)
                    nc.vector.tensor_copy(rzTs[:], rztp[:])
                    rzb = zpool.tile([P, CH, W], BF16, tag="rzb", bufs=4)
                    for c in range(CH):
                        rzbp = ps_zb.tile([P, W], F32, tag="zb")
                        nc.tensor.matmul(
                            rzbp[:],
                            lhsT=esel_sb[:, c, :],
                            rhs=rzTs[:],
                            start=True,
                            stop=True,
                        )
                        nc.vector.tensor_copy(rzb[:, c, :], rzbp[:])
                    rzbs[g] = rzb

                def av_yt(g, cs):
                    k0 = g * SC
                    rzb = rzbs[g]
                    for c in cs:
                        t = c // 2
                        ut4 = ut4s[(g, t)]
                        yp = ps_mm.tile([P, 512], F32, tag="mm")
                        for hf in range(2):
                            h = 2 * c + hf
                            nc.tensor.matmul(
                                yp[hf * HD : (hf + 1) * HD, :W],
                                lhsT=vtok[:SC, g, h * HD : (h + 1) * HD],
                                rhs=ut4[:, h % HT, :],
                                start=True,
                                stop=True,
                            )
                        nc.vector.tensor_tensor(
                            yT[:, c, k0 + 1 : k0 + SC],
                            yp[:, :W],
                            rzb[:, c, :],
                            mybir.AluOpType.mult,
                        )

                # ---- phase A: qk pipelined with scores; deferred prev proj ----
                for o in (0, 1, CH, CH + 1):
                    qk_group(0, o)
                ob_prev = None
                if yT_prev is not None:
                    ob_prev = opool.tile([P, CH, S], BF16, tag="ob")
                    proj_units = [
                        (o, n0, nsz) for o in range(CH) for (n0, nsz) in NSPLIT
                    ]
                zq = []
                for t in (1, 2):
                    sg = 0
                    for o in (2 * t, 2 * t + 1, CH + 2 * t, CH + 2 * t + 1):
                        qk_group(t, o)
                        for _ in range(2):
                            if sg < G:
                                score_group(t - 1, sg)
                                zq.append((sg, t - 1))
                                sg += 1
                            if len(zq) >= 2:
                                z_mms(*zq.pop(0))
                while zq:
                    z_mms(*zq.pop(0))

                # cls logits + exp. qcls[:, c, j] holds the cls query of head
                # 2c+j on that head's 64 partitions and zeros elsewhere, so one
                # [128, SC]-contract matmul yields two heads' logits at once.
                qcls = zpool.tile([P, CH, 2], BF16, tag="qcls", bufs=2)
                nc.vector.memset(qcls[:], 0.0)
                nc.vector.tensor_copy(qcls[0:HD, :, 0], qT[0:HD, :, 0])
                nc.vector.tensor_copy(qcls[HD:P, :, 1], qT[HD:P, :, 0])
                ucts = []
                for g in range(G):
                    cp = ps_zb.tile([SC, NH], F32, tag="zb")
                    for c in range(CH):
                        nc.tensor.matmul(
                            cp[:, 2 * c : 2 * c + 2],
                            lhsT=kT[:, c, g * SC : (g + 1) * SC],
                            rhs=qcls[:, c, :],
                            start=True,
                            stop=True,
                            skip_group_check=True,
                        )
                    uct = apool.tile([SC, NH], BF16, tag="uct", bufs=9)
                    nc.scalar.activation(
                        uct[:], cp[:], mybir.ActivationFunctionType.Exp
                    )
                    if g > 0:
                        nc.vector.memset(uct[0:1, :], 0.0)
                    ucts.append(uct)

                # ---- v-projection stretch: v + t=2 scores + rz + AV, lagged ----
                for g in range(G):
                    v_split(g, *VSPLIT[0])
                    if ob_prev is not None and proj_units and g >= 1:
                        proj_unit(yT_prev, ob_prev, *proj_units.pop(0))
                    if g >= 1:
                        rz_chain(g - 1)
                    if g >= 2:
                        av_yt(g - 2, range(0, 3))
                    v_split(g, *VSPLIT[1])
                    if ob_prev is not None and proj_units and g >= 2:
                        proj_unit(yT_prev, ob_prev, *proj_units.pop(0))
                    if g >= 2:
                        av_yt(g - 2, range(3, CH))
                    score_group(2, g)
                rz_chain(G - 1)
                av_yt(G - 2, range(CH))
                if ob_prev is not None:
                    while proj_units:
                        proj_unit(yT_prev, ob_prev, *proj_units.pop(0))
                    nc.sync.dma_start(
                        out_t[:, (b - 1) * S : b * S].rearrange(
                            "(c p) s -> p c s", p=P
                        ),
                        ob_prev[:],
                    )
                av_yt(G - 1, range(CH))

                # ---- cls Z + normalize + AV ----
                zp = ps_zb.tile([1, NH], F32, tag="zb")
                for g in range(G):
                    nc.tensor.matmul(
                        zp[:],
                        lhsT=onesp[:SC, :],
                        rhs=ucts[g][:],
                        start=(g == 0),
                        stop=(g == G - 1),
                    )
                zcs = zpool.tile([1, NH], F32, tag="zcs")
                nc.vector.tensor_copy(zcs[:], zp[:])
                rzc = zpool.tile([1, NH], F32, tag="rzc")
                nc.vector.reciprocal(rzc[:], zcs[:])
                rzcb = ps_zb.tile([P, NH], F32, tag="zb")
                nc.tensor.matmul(
                    rzcb[:], lhsT=onesrow[:], rhs=rzc[:], start=True, stop=True
                )
                for g in range(G):
                    nc.vector.tensor_tensor(
                        ucts[g][:], ucts[g][:], rzcb[:SC, :], mybir.AluOpType.mult
                    )
                for c in range(CH):
                    yp = ps_mm.tile([P, 512], F32, tag="mm")
                    for g in range(G):
                        nc.tensor.matmul(
                            yp[:, 0:2],
                            lhsT=vtok[:SC, g, 2 * c * HD : (2 * c + 2) * HD],
                            rhs=ucts[g][:, 2 * c : 2 * c + 2],
                            start=(g == 0),
                            stop=(g == G - 1),
                        )
                    # diagonal halves: head 2c lives in col 0 rows 0:64,
                    # head 2c+1 in col 1 rows 64:128
                    nc.vector.tensor_copy(
                        yT[0:HD, c, 0 : S : SC],
                        yp[0:HD, 0:1].to_broadcast([HD, G]),
                    )
                    nc.vector.tensor_copy(
                        yT[HD:P, c, 0 : S : SC],
                        yp[HD:P, 1:2].to_broadcast([HD, G]),
                    )

                yT_prev = yT

            proj_batch(NB - 1, yT_prev)

    nc.compile()
    return nc


_NC_CACHE = None
_LAST_IN_MAPS = None


def kernel(x, w_qkv, w_proj, b_proj):
    global _NC_CACHE, _LAST_IN_MAPS
    x = np.asarray(x)
    w_qkv = np.asarray(w_qkv)
    w_proj = np.asarray(w_proj)
    b_proj = np.asarray(b_proj)

    perm, valid = _perm_valid()
    maskt, esel = _consts()

    wq = np.array(w_qkv, np.float32, copy=True)
    wq[:, :C] *= 1.0 / np.sqrt(HD)
    wq = wq.astype(BFNP)
    wp = w_proj.astype(BFNP)
    b_pc = np.ascontiguousarray(b_proj.astype(np.float32).reshape(CH, P).T)

    in_maps = []
    for core in range(NCORES):
        xs = x[core * NB : (core + 1) * NB]          # (NB, 785, C)
        xp = xs[:, perm, :]                          # (NB, S, C)
        x_T = np.ascontiguousarray(
            xp.transpose(2, 0, 1).reshape(C, TT)
        ).astype(BFNP)
        in_maps.append(
            {
                "x_t": x_T,
                "w_qkv": wq,
                "w_proj": wp,
                "b_pc": b_pc,
                "maskt": maskt,
                "esel": esel,
            }
        )

    if _NC_CACHE is None:
        _NC_CACHE = build_bass()
    nc = _NC_CACHE

    _LAST_IN_MAPS = in_maps

    res = run_bass_kernel_spmd(nc, in_maps, core_ids=list(range(NCORES)))

    out = np.zeros((B_TOTAL, N_TOK, C), np.float32)
    vperm = perm[valid]
    for core in range(NCORES):
        o_t = np.asarray(res.results[core]["out_t"], dtype=np.float32)  # (C, TT)
        op = o_t.reshape(C, NB, S).transpose(1, 2, 0)  # (NB, S, C)
        out[core * NB : (core + 1) * NB][:, vperm, :] = op[:, valid, :]
    return out


if __name__ == "__main__":
    rng = np.random.default_rng(0)
    x = rng.standard_normal((B_TOTAL, N_TOK, C)).astype(np.float32)
    w_qkv = (rng.standard_normal((C, 3 * C)) * 0.02).astype(np.float32)
    w_proj = (rng.standard_normal((C, C)) * 0.02).astype(np.float32)
    b_proj = np.zeros((C,), np.float32)
    y = kernel(x=x, w_qkv=w_qkv, w_proj=w_proj, b_proj=b_proj)
    print(y.shape, y.dtype)


# revision 32
# speedup vs baseline: 1.1783x; 1.0141x over previous
"""AxialAttention (vertical, cls token, full cls attention) Trainium2 kernel.

Strategy: data-parallel over batch (32 batches -> 8 cores x 4 batches).
Per core everything is done in one fused Bass/Tile program:
  qkv projection -> per-row axial attention (+ full cls attention) -> out proj.

Host-side layout (per batch element):
  tokens are regrouped into 7 "slots" of 113 columns: [cls, 4 rows x 28 tok].
  Everything on-chip is feature-major (C on partitions): x_T (768, 791*4).

Row attention computes scores TRANSPOSED (keys on partitions) so no PE
transpose of the attention weights is needed:
  scoresT[k, q] = k_slot.T @ q_slot per head, 4 heads per PSUM tile with one
  bank per head (HW requires one matmul group per 2KB PSUM bank), one exp per
  4-head tile straight to SBUF, one multiplicative-mask multiply (exp(s+m-30)
  == exp(s)*M with M=1 valid / e^-30 invalid), per-head Z via tiny ones-matmuls
  into a shared [112, 84] PSUM tile, reciprocal, and a per-(slot, chunk)
  PE-broadcast of 1/Z (eye2-matmul) applied at the yT writeback
  (tensor_tensor multiply instead of a copy).
cls attention result is broadcast into all 7 dup-cls columns of yT so no
yT memset is needed.

Emission is software-pipelined per batch so each engine gets long runs of
independent work: qk chunk-pairs interleaved with score tiles, the previous
batch's projection as dense cover, then a v-projection stretch carrying the
t=2 scores, the 1/Z chains and the AV+writeback with 1-2 slot lag.
"""

import os

os.environ.setdefault("JAX_PLATFORMS", "axon")

import sys

if "/opt/trn_rl_repo" not in sys.path:
    sys.path.insert(0, "/opt/trn_rl_repo")

import numpy as np
import ml_dtypes

import concourse.bass as bass
import concourse.bacc as bacc
import concourse.mybir as mybir
import concourse.tile as tile
from concourse.bass_utils import run_bass_kernel_spmd
from concourse.masks import make_identity

P = 128
C = 768
CH = C // P            # 6 feature chunks
NH = 12
HD = 64
HH = 28                # image H = W
ROWS = 28              # attention rows per image
RG = 4                 # rows per slot
G = 7                  # slots per batch
W = RG * HH            # 112 queries per slot
SC = W + 1             # 113 keys per slot (cls + tokens)
S = G * SC             # 791 columns per batch
NB = 4                 # batches per core
TT = NB * S            # 3164 columns per core
NCORES = 8
B_TOTAL = 32
N_TOK = 1 + ROWS * HH  # 785
MPEN = 30.0            # mask penalty
HT = 4                 # heads per score tile
NT = NH // HT          # head tiles (3)
BANK = 512             # f32 elements per PSUM bank

F32 = mybir.dt.float32
BF16 = mybir.dt.bfloat16
BFNP = ml_dtypes.bfloat16


def _perm_valid():
    """original-token index for each of the S slot-layout columns + validity."""
    perm = np.zeros(S, np.int64)
    valid = np.ones(S, np.bool_)
    for g in range(G):
        perm[g * SC] = 0
        if g > 0:
            valid[g * SC] = False
        for j in range(W):
            r = RG * g + j // HH   # row index (original column w)
            i = j % HH             # position in row (original row h)
            perm[g * SC + 1 + j] = 1 + i * HH + r
    return perm, valid


def _consts():
    # multiplicative softmax mask, TRANSPOSED ([key, query]): exp(s + m - 30)
    # == exp(s) * M with M = 1 (cls key or same-row key) else e^-30
    m = np.full((SC, W), np.exp(-MPEN), np.float32)
    m[0, :] = 1.0
    for q in range(W):
        r = q // HH
        m[1 + r * HH : 1 + (r + 1) * HH, q] = 1.0
    # esel[h, c*128+p] = 1 iff h == 2c + (p >= 64): per-chunk selector that
    # broadcasts two heads' 1/Z rows onto the two 64-partition halves
    es = np.zeros((NH, CH * P), np.float32)
    for c in range(CH):
        es[2 * c, c * P : c * P + HD] = 1.0
        es[2 * c + 1, c * P + HD : (c + 1) * P] = 1.0
    return m.astype(BFNP), es.astype(BFNP)


def build_bass():
    nc = bacc.Bacc(None, target_bir_lowering=False, debug=True)

    x_t = nc.declare_dram_parameter("x_t", [C, TT], BF16, isOutput=False)
    w_qkv = nc.declare_dram_parameter("w_qkv", [C, 3 * C], BF16, isOutput=False)
    w_proj = nc.declare_dram_parameter("w_proj", [C, C], BF16, isOutput=False)
    b_pc = nc.declare_dram_parameter("b_pc", [P, CH], F32, isOutput=False)
    maskt = nc.declare_dram_parameter("maskt", [SC, W], BF16, isOutput=False)
    esel = nc.declare_dram_parameter("esel", [NH, CH * P], BF16, isOutput=False)
    out_t = nc.declare_dram_parameter("out_t", [C, TT], BF16, isOutput=True)

    NSPLIT = [(0, 512), (512, S - 512)]          # moving-dim tiling of S
    VSPLIT = [(0, 512), (512, C - 512)]          # moving-dim tiling of C (v cols)

    with tile.TileContext(nc) as tc:
        with (
            tc.tile_pool(name="const", bufs=1) as cpool,
            tc.tile_pool(name="xb", bufs=2) as xpool,
            tc.tile_pool(name="qk", bufs=2) as qkpool,
            tc.tile_pool(name="vt", bufs=2) as vpool,
            tc.tile_pool(name="yt", bufs=2) as ypool,
            tc.tile_pool(name="ob", bufs=2) as opool,
            tc.tile_pool(name="att", bufs=3) as apool,
            tc.tile_pool(name="attz", bufs=2) as zpool,
            tc.tile_pool(name="ps_mm", bufs=2, space="PSUM") as ps_mm,
            tc.tile_pool(name="ps_sc", bufs=1, space="PSUM") as ps_sc,
            tc.tile_pool(name="ps_z", bufs=1, space="PSUM") as ps_z,
            tc.tile_pool(name="ps_zb", bufs=1, space="PSUM") as ps_zb,
        ):
            # ---- constants ----
            wq_sb = cpool.tile([P, CH, 3 * C], BF16)
            nc.sync.dma_start(wq_sb[:], w_qkv.rearrange("(c p) o -> p c o", p=P))
            wp_sb = cpool.tile([P, CH, C], BF16)
            nc.sync.dma_start(wp_sb[:], w_proj.rearrange("(c p) o -> p c o", p=P))
            b_sb = cpool.tile([P, CH], F32)
            nc.sync.dma_start(b_sb[:], b_pc[:])
            mask_sb = cpool.tile([SC, W], BF16)
            nc.sync.dma_start(mask_sb[:], maskt[:])
            esel_sb = cpool.tile([NH, CH, P], BF16)
            nc.sync.dma_start(esel_sb[:], esel.rearrange("h (c p) -> h c p", p=P))
            ident = cpool.tile([W, W], F32)
            make_identity(nc, ident[:])
            onesp = cpool.tile([P, 1], BF16)
            nc.vector.memset(onesp[:], 1.0)
            onesrow = cpool.tile([1, P], F32)
            nc.vector.memset(onesrow[:], 1.0)

            def proj_unit(yTp, ob, o, n0, nsz):
                ps = ps_mm.tile([P, 512], F32, tag="mm")
                for c in range(CH):
                    nc.tensor.matmul(
                        ps[:, :nsz],
                        lhsT=wp_sb[:, c, o * P : (o + 1) * P],
                        rhs=yTp[:, c, n0 : n0 + nsz],
                        start=(c == 0),
                        stop=(c == CH - 1),
                    )
                nc.scalar.activation(
                    ob[:, o, n0 : n0 + nsz],
                    ps[:, :nsz],
                    mybir.ActivationFunctionType.Identity,
                    bias=b_sb[:, o : o + 1],
                )

            def proj_batch(pb, yTp):
                ob = opool.tile([P, CH, S], BF16, tag="ob")
                for o in range(CH):
                    for (n0, nsz) in NSPLIT:
                        proj_unit(yTp, ob, o, n0, nsz)
                nc.sync.dma_start(
                    out_t[:, pb * S : (pb + 1) * S].rearrange(
                        "(c p) s -> p c s", p=P
                    ),
                    ob[:],
                )

            yT_prev = None
            for b in range(NB):
                cb = b * S
                xb = xpool.tile([P, CH, S], BF16, tag="xb")
                nc.sync.dma_start(
                    xb[:], x_t[:, cb : cb + S].rearrange("(c p) s -> p c s", p=P)
                )

                qT = qkpool.tile([P, CH, S], BF16, tag="qT")
                kT = qkpool.tile([P, CH, S], BF16, tag="kT")
                vtok = vpool.tile([P, G, C], BF16, tag="vtok")
                yT = ypool.tile([P, CH, S], BF16, tag="yT")
                # per-head Z for all slots: column NH*g+h (one 2KB bank)
                zall = ps_z.tile([W, G * NH], F32, tag="z")
                rzt = zpool.tile([W, G * NH], F32, tag="rzt", bufs=2)

                ut4s = {}
                rzbs = {}

                def qk_group(t, o):
                    dst = qT if o < CH else kT
                    oc = o % CH
                    for si, (n0, nsz) in enumerate(NSPLIT):
                        ps = ps_mm.tile([P, 512], F32, tag="mm")
                        for c in range(CH):
                            nc.tensor.matmul(
                                ps[:, :nsz],
                                lhsT=wq_sb[:, c, o * P : (o + 1) * P],
                                rhs=xb[:, c, n0 : n0 + nsz],
                                start=(c == 0),
                                stop=(c == CH - 1),
                            )
                        if si == 0:
                            nc.scalar.copy(dst[:, oc, n0 : n0 + nsz], ps[:, :nsz])
                        else:
                            nc.vector.tensor_copy(
                                dst[:, oc, n0 : n0 + nsz], ps[:, :nsz]
                            )

                def score_group(t, g):
                    # transposed scores: one matmul per head per PSUM bank
                    k0 = g * SC
                    sp = ps_sc.tile([SC, HT, BANK], F32, tag="sc")
                    for hh in range(HT):
                        h = HT * t + hh
                        c, hf = h // 2, (h % 2) * HD
                        nc.tensor.matmul(
                            sp[:, hh, :W],
                            lhsT=kT[hf : hf + HD, c, k0 : k0 + SC],
                            rhs=qT[hf : hf + HD, c, k0 + 1 : k0 + SC],
                            start=True,
                            stop=True,
                        )
                    ut4 = apool.tile([SC, HT, W], BF16, tag="ut", bufs=24)
                    nc.scalar.activation(
                        ut4[:], sp[:, :, :W], mybir.ActivationFunctionType.Exp
                    )
                    nc.vector.tensor_tensor(
                        ut4[:],
                        ut4[:],
                        mask_sb[:].rearrange("k q -> k () q").to_broadcast(
                            [SC, HT, W]
                        ),
                        mybir.AluOpType.mult,
                    )
                    ut4s[(g, t)] = ut4

                def v_split(g, v0, vsz):
                    ps = ps_mm.tile([P, 512], F32, tag="mm")
                    for c in range(CH):
                        nc.tensor.matmul(
                            ps[:SC, :vsz],
                            lhsT=xb[:, c, g * SC : (g + 1) * SC],
                            rhs=wq_sb[:, c, 2 * C + v0 : 2 * C + v0 + vsz],
                            start=(c == 0),
                            stop=(c == CH - 1),
                        )
                    nc.scalar.copy(vtok[:SC, g, v0 : v0 + vsz], ps[:SC, :vsz])

                def z_mms(g, t):
                    # per-head Z via tiny ones-matmuls into the shared bank
                    ut4 = ut4s[(g, t)]
                    for hh in range(HT):
                        col = NH * g + HT * t + hh
                        nc.tensor.matmul(
                            zall[:, col : col + 1],
                            lhsT=ut4[:, hh, :],
                            rhs=onesp[:SC, :],
                            start=True,
                            stop=True,
                        )

                def rz_chain(g):
                    # 1/Z then broadcast each head's row onto its
                    # 64-partition half via GpSimd
                    z_mms(g, 2)
                    nc.vector.reciprocal(
                        rzt[:, NH * g : NH * (g + 1)],
                        zall[:, NH * g : NH * (g + 1)],
                    )
                    rztp = ps_zb.tile([NH, W], F32, tag="zb")
                    nc.tensor.transpose(
                        rztp[:], rzt[:, NH * g : NH * (g + 1)], ident[:]
                    )
                    rzTs = zpool.tile([NH, W], BF16, tag="rzTs", bufs=5)
                    nc.vector.tensor_copy(rzTs[:], rztp[:])
                    rzb = zpool.tile([P, CH, W], BF16, tag="rzb", bufs=6)
                    for c in range(CH):
                        rzbp = ps_zb.tile([P, W], F32, tag="zb")
                        nc.tensor.matmul(
                            rzbp[:],
                            lhsT=esel_sb[:, c, :],
                            rhs=rzTs[:],
                            start=True,
                            stop=True,
                        )
                        nc.vector.tensor_copy(rzb[:, c, :], rzbp[:])
                    rzbs[g] = rzb

                def av_yt(g, cs):
                    k0 = g * SC
                    rzb = rzbs[g]
                    for c in cs:
                        t = c // 2
                        ut4 = ut4s[(g, t)]
                        yp = ps_mm.tile([P, 512], F32, tag="mm")
                        for hf in range(2):
                            h = 2 * c + hf
                            nc.tensor.matmul(
                                yp[hf * HD : (hf + 1) * HD, :W],
                                lhsT=vtok[:SC, g, h * HD : (h + 1) * HD],
                                rhs=ut4[:, h % HT, :],
                                start=True,
                                stop=True,
                            )
                        nc.vector.tensor_tensor(
                            yT[:, c, k0 + 1 : k0 + SC],
                            yp[:, :W],
                            rzb[:, c, :],
                            mybir.AluOpType.mult,
                        )

                # ---- phase A: qk pipelined with scores; deferred prev proj ----
                for o in (0, 1, CH, CH + 1):
                    qk_group(0, o)
                ob_prev = None
                if yT_prev is not None:
                    ob_prev = opool.tile([P, CH, S], BF16, tag="ob")
                    proj_units = [
                        (o, n0, nsz) for o in range(CH) for (n0, nsz) in NSPLIT
                    ]
                zq = []
                for t in (1, 2):
                    sg = 0
                    for o in (2 * t, 2 * t + 1, CH + 2 * t, CH + 2 * t + 1):
                        qk_group(t, o)
                        for _ in range(2):
                            if sg < G:
                                score_group(t - 1, sg)
                                zq.append((sg, t - 1))
                                sg += 1
                            if len(zq) >= 2:
                                z_mms(*zq.pop(0))
                while zq:
                    z_mms(*zq.pop(0))

                # cls logits + exp. qcls[:, c, j] holds the cls query of head
                # 2c+j on that head's 64 partitions and zeros elsewhere, so one
                # [128, SC]-contract matmul yields two heads' logits at once.
                qcls = zpool.tile([P, CH, 2], BF16, tag="qcls", bufs=2)
                nc.vector.memset(qcls[:], 0.0)
                nc.vector.tensor_copy(qcls[0:HD, :, 0], qT[0:HD, :, 0])
                nc.vector.tensor_copy(qcls[HD:P, :, 1], qT[HD:P, :, 0])
                ucts = []
                for g in range(G):
                    cp = ps_zb.tile([SC, NH], F32, tag="zb")
                    for c in range(CH):
                        nc.tensor.matmul(
                            cp[:, 2 * c : 2 * c + 2],
                            lhsT=kT[:, c, g * SC : (g + 1) * SC],
                            rhs=qcls[:, c, :],
                            start=True,
                            stop=True,
                            skip_group_check=True,
                        )
                    uct = apool.tile([SC, NH], BF16, tag="uct", bufs=9)
                    nc.scalar.activation(
                        uct[:], cp[:], mybir.ActivationFunctionType.Exp
                    )
                    if g > 0:
                        nc.vector.memset(uct[0:1, :], 0.0)
                    ucts.append(uct)

                # ---- v-projection stretch: v + t=2 scores + rz + AV, lagged ----
                for g in range(G):
                    v_split(g, *VSPLIT[0])
                    if ob_prev is not None and proj_units and g >= 1:
                        proj_unit(yT_prev, ob_prev, *proj_units.pop(0))
                    if g >= 1:
                        rz_chain(g - 1)
                    if g >= 2:
                        av_yt(g - 2, range(0, 3))
                    v_split(g, *VSPLIT[1])
                    if ob_prev is not None and proj_units and g >= 2:
                        proj_unit(yT_prev, ob_prev, *proj_units.pop(0))
                    if g >= 2:
                        av_yt(g - 2, range(3, CH))
                    score_group(2, g)
                rz_chain(G - 1)
                av_yt(G - 2, range(CH))
                if ob_prev is not None:
                    while proj_units:
                        proj_unit(yT_prev, ob_prev, *proj_units.pop(0))
                    nc.sync.dma_start(
                        out_t[:, (b - 1) * S : b * S].rearrange(
                            "(c p) s -> p c s", p=P
                        ),
                        ob_prev[:],
                    )
                av_yt(G - 1, range(CH))

                # ---- cls Z + normalize + AV ----
                zp = ps_zb.tile([1, NH], F32, tag="zb")
                for g in range(G):
                    nc.tensor.matmul(
                        zp[:],
                        lhsT=onesp[:SC, :],
                        rhs=ucts[g][:],
                        start=(g == 0),
                        stop=(g == G - 1),
                    )
                zcs = zpool.tile([1, NH], F32, tag="zcs")
                nc.vector.tensor_copy(zcs[:], zp[:])
                rzc = zpool.tile([1, NH], F32, tag="rzc")
                nc.vector.reciprocal(rzc[:], zcs[:])
                rzcb = ps_zb.tile([P, NH], F32, tag="zb")
                nc.tensor.matmul(
                    rzcb[:], lhsT=onesrow[:], rhs=rzc[:], start=True, stop=True
                )
                for g in range(G):
                    nc.vector.tensor_tensor(
                        ucts[g][:], ucts[g][:], rzcb[:SC, :], mybir.AluOpType.mult
                    )
                for c in range(CH):
                    yp = ps_mm.tile([P, 512], F32, tag="mm")
                    for g in range(G):
                        nc.tensor.matmul(
                            yp[:, 0:2],
                            lhsT=vtok[:SC, g, 2 * c * HD : (2 * c + 2) * HD],
                            rhs=ucts[g][:, 2 * c : 2 * c + 2],
                            start=(g == 0),
                            stop=(g == G - 1),
                        )
                    # diagonal halves: head 2c lives in col 0 rows 0:64,
                    # head 2c+1 in col 1 rows 64:128
                    nc.vector.tensor_copy(
                        yT[0:HD, c, 0 : S : SC],
                        yp[0:HD, 0:1].to_broadcast([HD, G]),
                    )
                    nc.vector.tensor_copy(
                        yT[HD:P, c, 0 : S : SC],
                        yp[HD:P, 1:2].to_broadcast([HD, G]),
                    )

                yT_prev = yT

            proj_batch(NB - 1, yT_prev)

    nc.compile()
    return nc


_NC_CACHE = None
_LAST_IN_MAPS = None


def kernel(x, w_qkv, w_proj, b_proj):
    global _NC_CACHE, _LAST_IN_MAPS
    x = np.asarray(x)
    w_qkv = np.asarray(w_qkv)
    w_proj = np.asarray(w_proj)
    b_proj = np.asarray(b_proj)

    perm, valid = _perm_valid()
    maskt, esel = _consts()

    wq = np.array(w_qkv, np.float32, copy=True)
    wq[:, :C] *= 1.0 / np.sqrt(HD)
    wq = wq.astype(BFNP)
    wp = w_proj.astype(BFNP)
    b_pc = np.ascontiguousarray(b_proj.astype(np.float32).reshape(CH, P).T)

    in_maps = []
    for core in range(NCORES):
        xs = x[core * NB : (core + 1) * NB]          # (NB, 785, C)
        xp = xs[:, perm, :]                          # (NB, S, C)
        x_T = np.ascontiguousarray(
            xp.transpose(2, 0, 1).reshape(C, TT)
        ).astype(BFNP)
        in_maps.append(
            {
                "x_t": x_T,
                "w_qkv": wq,
                "w_proj": wp,
                "b_pc": b_pc,
                "maskt": maskt,
                "esel": esel,
            }
        )

    if _NC_CACHE is None:
        _NC_CACHE = build_bass()
    nc = _NC_CACHE

    _LAST_IN_MAPS = in_maps

    res = run_bass_kernel_spmd(nc, in_maps, core_ids=list(range(NCORES)))

    out = np.zeros((B_TOTAL, N_TOK, C), np.float32)
    vperm = perm[valid]
    for core in range(NCORES):
        o_t = np.asarray(res.results[core]["out_t"], dtype=np.float32)  # (C, TT)
        op = o_t.reshape(C, NB, S).transpose(1, 2, 0)  # (NB, S, C)
        out[core * NB : (core + 1) * NB][:, vperm, :] = op[:, valid, :]
    return out


if __name__ == "__main__":
    rng = np.random.default_rng(0)
    x = rng.standard_normal((B_TOTAL, N_TOK, C)).astype(np.float32)
    w_qkv = (rng.standard_normal((C, 3 * C)) * 0.02).astype(np.float32)
    w_proj = (rng.standard_normal((C, C)) * 0.02).astype(np.float32)
    b_proj = np.zeros((C,), np.float32)
    y = kernel(x=x, w_qkv=w_qkv, w_proj=w_proj, b_proj=b_proj)
    print(y.shape, y.dtype)


# revision 35
# speedup vs baseline: 1.1816x; 1.0028x over previous
"""AxialAttention (vertical, cls token, full cls attention) Trainium2 kernel.

Strategy: data-parallel over batch (32 batches -> 8 cores x 4 batches).
Per core everything is done in one fused Bass/Tile program:
  qkv projection -> per-row axial attention (+ full cls attention) -> out proj.

Host-side layout (per batch element):
  tokens are regrouped into 7 "slots" of 113 columns: [cls, 4 rows x 28 tok].
  Everything on-chip is feature-major (C on partitions): x_T (768, 791*4).

Row attention computes scores TRANSPOSED (keys on partitions) so no PE
transpose of the attention weights is needed:
  scoresT[k, q] = k_slot.T @ q_slot per head, 4 heads per PSUM tile with one
  bank per head (HW requires one matmul group per 2KB PSUM bank), one exp per
  4-head tile straight to SBUF, one multiplicative-mask multiply (exp(s+m-30)
  == exp(s)*M with M=1 valid / e^-30 invalid), per-head Z via tiny ones-matmuls
  into a shared [112, 84] PSUM tile, reciprocal, and a per-(slot, chunk)
  PE-broadcast of 1/Z (eye2-matmul) applied at the yT writeback
  (tensor_tensor multiply instead of a copy).
cls attention result is broadcast into all 7 dup-cls columns of yT so no
yT memset is needed.

Emission is software-pipelined per batch so each engine gets long runs of
independent work: qk chunk-pairs interleaved with score tiles, the previous
batch's projection as dense cover, then a v-projection stretch carrying the
t=2 scores, the 1/Z chains and the AV+writeback with 1-2 slot lag.
"""

import os

os.environ.setdefault("JAX_PLATFORMS", "axon")

import sys

if "/opt/trn_rl_repo" not in sys.path:
    sys.path.insert(0, "/opt/trn_rl_repo")

import numpy as np
import ml_dtypes

import concourse.bass as bass
import concourse.bacc as bacc
import concourse.mybir as mybir
import concourse.tile as tile
from concourse.bass_utils import run_bass_kernel_spmd
from concourse.masks import make_identity

P = 128
C = 768
CH = C // P            # 6 feature chunks
NH = 12
HD = 64
HH = 28                # image H = W
ROWS = 28              # attention rows per image
RG = 4                 # rows per slot
G = 7                  # slots per batch
W = RG * HH            # 112 queries per slot
SC = W + 1             # 113 keys per slot (cls + tokens)
S = G * SC             # 791 columns per batch
NB = 4                 # batches per core
TT = NB * S            # 3164 columns per core
NCORES = 8
B_TOTAL = 32
N_TOK = 1 + ROWS * HH  # 785
MPEN = 30.0            # mask penalty
HT = 4                 # heads per score tile
NT = NH // HT          # head tiles (3)
BANK = 512             # f32 elements per PSUM bank

F32 = mybir.dt.float32
BF16 = mybir.dt.bfloat16
BFNP = ml_dtypes.bfloat16


def _perm_valid():
    """original-token index for each of the S slot-layout columns + validity."""
    perm = np.zeros(S, np.int64)
    valid = np.ones(S, np.bool_)
    for g in range(G):
        perm[g * SC] = 0
        if g > 0:
            valid[g * SC] = False
        for j in range(W):
            r = RG * g + j // HH   # row index (original column w)
            i = j % HH             # position in row (original row h)
            perm[g * SC + 1 + j] = 1 + i * HH + r
    return perm, valid


def _consts():
    # multiplicative softmax mask, TRANSPOSED ([key, query]): exp(s + m - 30)
    # == exp(s) * M with M = 1 (cls key or same-row key) else e^-30
    m = np.full((SC, W), np.exp(-MPEN), np.float32)
    m[0, :] = 1.0
    for q in range(W):
        r = q // HH
        m[1 + r * HH : 1 + (r + 1) * HH, q] = 1.0
    # esel[h, c*128+p] = 1 iff h == 2c + (p >= 64): per-chunk selector that
    # broadcasts two heads' 1/Z rows onto the two 64-partition halves
    es = np.zeros((NH, CH * P), np.float32)
    for c in range(CH):
        es[2 * c, c * P : c * P + HD] = 1.0
        es[2 * c + 1, c * P + HD : (c + 1) * P] = 1.0
    return m.astype(BFNP), es.astype(BFNP)


def build_bass():
    nc = bacc.Bacc(None, target_bir_lowering=False, debug=True)

    x_t = nc.declare_dram_parameter("x_t", [C, TT], BF16, isOutput=False)
    w_qkv = nc.declare_dram_parameter("w_qkv", [C, 3 * C], BF16, isOutput=False)
    w_proj = nc.declare_dram_parameter("w_proj", [C, C], BF16, isOutput=False)
    b_pc = nc.declare_dram_parameter("b_pc", [P, CH], F32, isOutput=False)
    maskt = nc.declare_dram_parameter("maskt", [SC, W], BF16, isOutput=False)
    esel = nc.declare_dram_parameter("esel", [NH, CH * P], BF16, isOutput=False)
    out_t = nc.declare_dram_parameter("out_t", [C, TT], BF16, isOutput=True)

    NSPLIT = [(0, 512), (512, S - 512)]          # moving-dim tiling of S
    VSPLIT = [(0, 512), (512, C - 512)]          # moving-dim tiling of C (v cols)

    with tile.TileContext(nc) as tc:
        with (
            tc.tile_pool(name="const", bufs=1) as cpool,
            tc.tile_pool(name="xb", bufs=2) as xpool,
            tc.tile_pool(name="qk", bufs=2) as qkpool,
            tc.tile_pool(name="vt", bufs=2) as vpool,
            tc.tile_pool(name="yt", bufs=2) as ypool,
            tc.tile_pool(name="ob", bufs=2) as opool,
            tc.tile_pool(name="att", bufs=3) as apool,
            tc.tile_pool(name="attz", bufs=2) as zpool,
            tc.tile_pool(name="ps_mm", bufs=2, space="PSUM") as ps_mm,
            tc.tile_pool(name="ps_sc", bufs=1, space="PSUM") as ps_sc,
            tc.tile_pool(name="ps_z", bufs=1, space="PSUM") as ps_z,
            tc.tile_pool(name="ps_zb", bufs=1, space="PSUM") as ps_zb,
        ):
            # ---- constants ----
            wq_sb = cpool.tile([P, CH, 3 * C], BF16)
            nc.sync.dma_start(wq_sb[:], w_qkv.rearrange("(c p) o -> p c o", p=P))
            wp_sb = cpool.tile([P, CH, C], BF16)
            nc.sync.dma_start(wp_sb[:], w_proj.rearrange("(c p) o -> p c o", p=P))
            b_sb = cpool.tile([P, CH], F32)
            nc.sync.dma_start(b_sb[:], b_pc[:])
            mask_sb = cpool.tile([SC, W], BF16)
            nc.sync.dma_start(mask_sb[:], maskt[:])
            esel_sb = cpool.tile([NH, CH, P], BF16)
            nc.sync.dma_start(esel_sb[:], esel.rearrange("h (c p) -> h c p", p=P))
            ident = cpool.tile([W, W], F32)
            make_identity(nc, ident[:])
            onesp = cpool.tile([P, 1], BF16)
            nc.vector.memset(onesp[:], 1.0)
            onesrow = cpool.tile([1, P], F32)
            nc.vector.memset(onesrow[:], 1.0)

            def proj_unit(yTp, ob, o, n0, nsz):
                ps = ps_mm.tile([P, 512], F32, tag="mm")
                for c in range(CH):
                    nc.tensor.matmul(
                        ps[:, :nsz],
                        lhsT=wp_sb[:, c, o * P : (o + 1) * P],
                        rhs=yTp[:, c, n0 : n0 + nsz],
                        start=(c == 0),
                        stop=(c == CH - 1),
                    )
                nc.scalar.activation(
                    ob[:, o, n0 : n0 + nsz],
                    ps[:, :nsz],
                    mybir.ActivationFunctionType.Identity,
                    bias=b_sb[:, o : o + 1],
                )

            def proj_batch(pb, yTp):
                ob = opool.tile([P, CH, S], BF16, tag="ob")
                for o in range(CH):
                    for (n0, nsz) in NSPLIT:
                        proj_unit(yTp, ob, o, n0, nsz)
                nc.sync.dma_start(
                    out_t[:, pb * S : (pb + 1) * S].rearrange(
                        "(c p) s -> p c s", p=P
                    ),
                    ob[:],
                )

            yT_prev = None
            for b in range(NB):
                cb = b * S
                xb = xpool.tile([P, CH, S], BF16, tag="xb")
                nc.sync.dma_start(
                    xb[:], x_t[:, cb : cb + S].rearrange("(c p) s -> p c s", p=P)
                )

                qT = qkpool.tile([P, CH, S], BF16, tag="qT")
                kT = qkpool.tile([P, CH, S], BF16, tag="kT")
                vtok = vpool.tile([P, G, C], BF16, tag="vtok")
                yT = ypool.tile([P, CH, S], BF16, tag="yT")
                # per-head Z for all slots: column NH*g+h (one 2KB bank)
                zall = ps_z.tile([W, G * NH], F32, tag="z")
                rzt = zpool.tile([W, G * NH], F32, tag="rzt", bufs=2)

                ut4s = {}
                rzbs = {}

                def qk_group(t, o):
                    dst = qT if o < CH else kT
                    oc = o % CH
                    for si, (n0, nsz) in enumerate(NSPLIT):
                        ps = ps_mm.tile([P, 512], F32, tag="mm")
                        for c in range(CH):
                            nc.tensor.matmul(
                                ps[:, :nsz],
                                lhsT=wq_sb[:, c, o * P : (o + 1) * P],
                                rhs=xb[:, c, n0 : n0 + nsz],
                                start=(c == 0),
                                stop=(c == CH - 1),
                            )
                        if si == 0:
                            nc.scalar.copy(dst[:, oc, n0 : n0 + nsz], ps[:, :nsz])
                        else:
                            nc.vector.tensor_copy(
                                dst[:, oc, n0 : n0 + nsz], ps[:, :nsz]
                            )

                def score_group(t, g):
                    # transposed scores: one matmul per head per PSUM bank
                    k0 = g * SC
                    sp = ps_sc.tile([SC, HT, BANK], F32, tag="sc")
                    for hh in range(HT):
                        h = HT * t + hh
                        c, hf = h // 2, (h % 2) * HD
                        nc.tensor.matmul(
                            sp[:, hh, :W],
                            lhsT=kT[hf : hf + HD, c, k0 : k0 + SC],
                            rhs=qT[hf : hf + HD, c, k0 + 1 : k0 + SC],
                            start=True,
                            stop=True,
                        )
                    ut4 = apool.tile([SC, HT, W], BF16, tag="ut", bufs=24)
                    nc.scalar.activation(
                        ut4[:], sp[:, :, :W], mybir.ActivationFunctionType.Exp
                    )
                    nc.vector.tensor_tensor(
                        ut4[:],
                        ut4[:],
                        mask_sb[:].rearrange("k q -> k () q").to_broadcast(
                            [SC, HT, W]
                        ),
                        mybir.AluOpType.mult,
                    )
                    ut4s[(g, t)] = ut4

                def v_split(g, v0, vsz):
                    ps = ps_mm.tile([P, 512], F32, tag="mm")
                    for c in range(CH):
                        nc.tensor.matmul(
                            ps[:SC, :vsz],
                            lhsT=xb[:, c, g * SC : (g + 1) * SC],
                            rhs=wq_sb[:, c, 2 * C + v0 : 2 * C + v0 + vsz],
                            start=(c == 0),
                            stop=(c == CH - 1),
                        )
                    nc.scalar.copy(vtok[:SC, g, v0 : v0 + vsz], ps[:SC, :vsz])

                def z_mms(g, t):
                    # per-head Z via tiny ones-matmuls into the shared bank
                    ut4 = ut4s[(g, t)]
                    for hh in range(HT):
                        col = NH * g + HT * t + hh
                        nc.tensor.matmul(
                            zall[:, col : col + 1],
                            lhsT=ut4[:, hh, :],
                            rhs=onesp[:SC, :],
                            start=True,
                            stop=True,
                        )

                def rz_chain(g):
                    # 1/Z then broadcast each head's row onto its
                    # 64-partition half via GpSimd
                    z_mms(g, 2)
                    nc.vector.reciprocal(
                        rzt[:, NH * g : NH * (g + 1)],
                        zall[:, NH * g : NH * (g + 1)],
                    )
                    rztp = ps_zb.tile([NH, W], F32, tag="zb")
                    nc.tensor.transpose(
                        rztp[:], rzt[:, NH * g : NH * (g + 1)], ident[:]
                    )
                    rzTs = zpool.tile([NH, W], BF16, tag="rzTs", bufs=5)
                    nc.vector.tensor_copy(rzTs[:], rztp[:])
                    rzb = zpool.tile([P, CH, W], BF16, tag="rzb", bufs=6)
                    for c in range(CH):
                        rzbp = ps_zb.tile([P, W], F32, tag="zb")
                        nc.tensor.matmul(
                            rzbp[:],
                            lhsT=esel_sb[:, c, :],
                            rhs=rzTs[:],
                            start=True,
                            stop=True,
                        )
                        nc.vector.tensor_copy(rzb[:, c, :], rzbp[:])
                    rzbs[g] = rzb

                def av_yt(g, cs):
                    k0 = g * SC
                    rzb = rzbs[g]
                    for c in cs:
                        t = c // 2
                        ut4 = ut4s[(g, t)]
                        yp = ps_mm.tile([P, 512], F32, tag="mm")
                        for hf in range(2):
                            h = 2 * c + hf
                            nc.tensor.matmul(
                                yp[hf * HD : (hf + 1) * HD, :W],
                                lhsT=vtok[:SC, g, h * HD : (h + 1) * HD],
                                rhs=ut4[:, h % HT, :],
                                start=True,
                                stop=True,
                            )
                        nc.vector.tensor_tensor(
                            yT[:, c, k0 + 1 : k0 + SC],
                            yp[:, :W],
                            rzb[:, c, :],
                            mybir.AluOpType.mult,
                        )

                # ---- phase A: qk pipelined with scores; deferred prev proj ----
                for o in (0, 1, CH, CH + 1):
                    qk_group(0, o)
                ob_prev = None
                if yT_prev is not None:
                    ob_prev = opool.tile([P, CH, S], BF16, tag="ob")
                    proj_units = [
                        (o, n0, nsz) for o in range(CH) for (n0, nsz) in NSPLIT
                    ]
                zq = []
                for t in (1, 2):
                    sg = 0
                    for o in (2 * t, 2 * t + 1, CH + 2 * t, CH + 2 * t + 1):
                        qk_group(t, o)
                        for _ in range(2):
                            if sg < G:
                                score_group(t - 1, sg)
                                zq.append((sg, t - 1))
                                sg += 1
                            if len(zq) >= 2:
                                z_mms(*zq.pop(0))
                while zq:
                    z_mms(*zq.pop(0))

                # cls logits + exp. qcls[:, c, j] holds the cls query of head
                # 2c+j on that head's 64 partitions and zeros elsewhere, so one
                # [128, SC]-contract matmul yields two heads' logits at once.
                qcls = zpool.tile([P, CH, 2], BF16, tag="qcls", bufs=2)
                nc.vector.memset(qcls[:], 0.0)
                nc.vector.tensor_copy(qcls[0:HD, :, 0], qT[0:HD, :, 0])
                nc.vector.tensor_copy(qcls[HD:P, :, 1], qT[HD:P, :, 0])
                ucts = []
                for g in range(G):
                    cp = ps_zb.tile([SC, NH], F32, tag="zb")
                    for c in range(CH):
                        nc.tensor.matmul(
                            cp[:, 2 * c : 2 * c + 2],
                            lhsT=kT[:, c, g * SC : (g + 1) * SC],
                            rhs=qcls[:, c, :],
                            start=True,
                            stop=True,
                            skip_group_check=True,
                        )
                    uct = apool.tile([SC, NH], BF16, tag="uct", bufs=9)
                    nc.scalar.activation(
                        uct[:], cp[:], mybir.ActivationFunctionType.Exp
                    )
                    if g > 0:
                        nc.vector.memset(uct[0:1, :], 0.0)
                    ucts.append(uct)

                # ---- v-projection stretch: v + t=2 scores + rz + AV, lagged ----
                for g in range(G):
                    v_split(g, *VSPLIT[0])
                    if ob_prev is not None and proj_units and g >= 1:
                        proj_unit(yT_prev, ob_prev, *proj_units.pop(0))
                    if g >= 1:
                        rz_chain(g - 1)
                    if g >= 2:
                        av_yt(g - 2, range(0, 3))
                    v_split(g, *VSPLIT[1])
                    if ob_prev is not None and proj_units and g >= 2:
                        proj_unit(yT_prev, ob_prev, *proj_units.pop(0))
                    if g >= 2:
                        av_yt(g - 2, range(3, CH))
                    score_group(2, g)
                rz_chain(G - 1)
                # ---- cls Z + normalize + AV ----
                zp = ps_zb.tile([1, NH], F32, tag="zb")
                for g in range(G):
                    nc.tensor.matmul(
                        zp[:],
                        lhsT=onesp[:SC, :],
                        rhs=ucts[g][:],
                        start=(g == 0),
                        stop=(g == G - 1),
                    )
                zcs = zpool.tile([1, NH], F32, tag="zcs")
                nc.vector.tensor_copy(zcs[:], zp[:])
                rzc = zpool.tile([1, NH], F32, tag="rzc")
                nc.vector.reciprocal(rzc[:], zcs[:])
                rzcb = ps_zb.tile([P, NH], F32, tag="zb")
                nc.tensor.matmul(
                    rzcb[:], lhsT=onesrow[:], rhs=rzc[:], start=True, stop=True
                )
                for g in range(G):
                    nc.vector.tensor_tensor(
                        ucts[g][:], ucts[g][:], rzcb[:SC, :], mybir.AluOpType.mult
                    )
                av_yt(G - 2, range(CH))
                if ob_prev is not None:
                    while proj_units:
                        proj_unit(yT_prev, ob_prev, *proj_units.pop(0))
                    nc.sync.dma_start(
                        out_t[:, (b - 1) * S : b * S].rearrange(
                            "(c p) s -> p c s", p=P
                        ),
                        ob_prev[:],
                    )
                av_yt(G - 1, range(CH))

                for c in range(CH):
                    yp = ps_mm.tile([P, 512], F32, tag="mm")
                    for g in range(G):
                        nc.tensor.matmul(
                            yp[:, 0:2],
                            lhsT=vtok[:SC, g, 2 * c * HD : (2 * c + 2) * HD],
                            rhs=ucts[g][:, 2 * c : 2 * c + 2],
                            start=(g == 0),
                            stop=(g == G - 1),
                        )
                    # diagonal halves: head 2c lives in col 0 rows 0:64,
                    # head 2c+1 in col 1 rows 64:128
                    nc.vector.tensor_copy(
                        yT[0:HD, c, 0 : S : SC],
                        yp[0:HD, 0:1].to_broadcast([HD, G]),
                    )
                    nc.vector.tensor_copy(
                        yT[HD:P, c, 0 : S : SC],
                        yp[HD:P, 1:2].to_broadcast([HD, G]),
                    )

                yT_prev = yT

            proj_batch(NB - 1, yT_prev)

    nc.compile()
    return nc


_NC_CACHE = None
_LAST_IN_MAPS = None


def kernel(x, w_qkv, w_proj, b_proj):
    global _NC_CACHE, _LAST_IN_MAPS
    x = np.asarray(x)
    w_qkv = np.asarray(w_qkv)
    w_proj = np.asarray(w_proj)
    b_proj = np.asarray(b_proj)

    perm, valid = _perm_valid()
    maskt, esel = _consts()

    wq = np.array(w_qkv, np.float32, copy=True)
    wq[:, :C] *= 1.0 / np.sqrt(HD)
    wq = wq.astype(BFNP)
    wp = w_proj.astype(BFNP)
    b_pc = np.ascontiguousarray(b_proj.astype(np.float32).reshape(CH, P).T)

    in_maps = []
    for core in range(NCORES):
        xs = x[core * NB : (core + 1) * NB]          # (NB, 785, C)
        xp = xs[:, perm, :]                          # (NB, S, C)
        x_T = np.ascontiguousarray(
            xp.transpose(2, 0, 1).reshape(C, TT)
        ).astype(BFNP)
        in_maps.append(
            {
                "x_t": x_T,
                "w_qkv": wq,
                "w_proj": wp,
                "b_pc": b_pc,
                "maskt": maskt,
                "esel": esel,
            }
        )

    if _NC_CACHE is None:
        _NC_CACHE = build_bass()
    nc = _NC_CACHE

    _LAST_IN_MAPS = in_maps

    res = run_bass_kernel_spmd(nc, in_maps, core_ids=list(range(NCORES)))

    out = np.zeros((B_TOTAL, N_TOK, C), np.float32)
    vperm = perm[valid]
    for core in range(NCORES):
        o_t = np.asarray(res.results[core]["out_t"], dtype=np.float32)  # (C, TT)
        op = o_t.reshape(C, NB, S).transpose(1, 2, 0)  # (NB, S, C)
        out[core * NB : (core + 1) * NB][:, vperm, :] = op[:, valid, :]
    return out


if __name__ == "__main__":
    rng = np.random.default_rng(0)
    x = rng.standard_normal((B_TOTAL, N_TOK, C)).astype(np.float32)
    w_qkv = (rng.standard_normal((C, 3 * C)) * 0.02).astype(np.float32)
    w_proj = (rng.standard_normal((C, C)) * 0.02).astype(np.float32)
    b_proj = np.zeros((C,), np.float32)
    y = kernel(x=x, w_qkv=w_qkv, w_proj=w_proj, b_proj=b_proj)
    print(y.shape, y.dtype)
